# revision 31
# baseline (speedup 1.0000x reference)
"""Trainium2 Bass kernel for CustomSelfAttention (B=8,S=1024,D=1024,H=16,K=64).

Strategy: data-parallel over batch across 8 NeuronCores (1 batch item/core).
All matmuls in bf16 (host pre-casts x and weights; 1 cycle/column on the PE
vs 2 for fp32, FWL weight loads). Per-core pipeline:
  0. xT [d, s] via hardware XBAR DMA-transpose of bf16 x (no PE transposes).
  1. qT = (Wq/8)^T x^T, kT = Wk^T x^T (layout [hk, s]); v = x Wv stored
     interleaved with a ones column per head: vext [s, h, 65].
  2. attention per head PAIR (even head on PE rows 0-63, odd on 64-127 via
     tile_position row tiling -> the two K=64 scores matmuls run
     concurrently); one Exp activation over a 2-bank PSUM tile [128,1024]
     with the per-partition key-mask bias fused; ctx matmuls with
     lhsT=[v_h | 1] give ctx^T[k,q] plus softmax row sums in one shot;
     normalize with reciprocal_approx_fast + partition_broadcast.
     QKV projection matmuls for chunk c+1 are interleaved into attention
     chunk c's PE queue to cover the ACT-bound exp latency.
  3. out = ctxT^T Wo + bo, blended with the uniform-attention row for
     fully-masked queries (softmax of a row of -1e9 is exactly uniform),
     computed as u = mean_s(v) Wo.
"""

import contextlib
import sys
import types

sys.path.insert(0, "/opt/trn_rl_repo")

# The image's antenv package may lack axon_hooks (NTFF profile hook
# registry); bass_utils imports it unconditionally when trace=True.
# Install a functional shim + register the ctypes hook like
# trn_agent_boot.trn_boot does.
try:
    import antenv.axon_hooks  # noqa: F401
except ImportError:
    try:
        import antenv

        _hooks_mod = types.ModuleType("antenv.axon_hooks")
        _hook_box = [None]
        _hooks_mod.get_axon_ntff_profile_hook = lambda: _hook_box[0]
        _hooks_mod.set_axon_ntff_profile_hook = (
            lambda h: _hook_box.__setitem__(0, h)
        )
        sys.modules["antenv.axon_hooks"] = _hooks_mod
        antenv.axon_hooks = _hooks_mod
        from trn_agent_boot.trn_boot import _ntff_profile_via_ctypes

        _hooks_mod.set_axon_ntff_profile_hook(
            _ntff_profile_via_ctypes("/opt/axon/libaxon_pjrt.so")
        )
    except Exception:
        pass

import ml_dtypes  # noqa: E402
import numpy as np  # noqa: E402

import concourse.bass as bass  # noqa: E402
import concourse.bass_utils as _bass_utils  # noqa: E402
import concourse.mybir as mybir  # noqa: E402
import concourse.tile as tile  # noqa: E402
from concourse import bacc  # noqa: E402
from concourse.bass_utils import run_bass_kernel_spmd  # noqa: E402
from concourse.masks import make_identity  # noqa: E402

# Enable the walrus LDWEIGHTS background-buffer optimization for this
# kernel's compile: without it every MATMUL serializes behind its
# foreground weight load (~+170ns per matmul on this kernel). Walrus
# rejects ldw-opt when LDWEIGHTS carry semaphore waits, so the bass pass
# that moves matmul waits onto LDWEIGHTS must be skipped too (see
# _build_nc).
LDW_OPT = False

if not getattr(_bass_utils, "_ldwopt_patched", False):
    _orig_run_command = _bass_utils.run_command

    def _run_command_ldwopt(argv, **kwargs):
        if LDW_OPT and isinstance(argv, list):
            argv = [
                "--enable-ldw-opt=true" if a == "--enable-ldw-opt=false" else a
                for a in argv
            ]
        return _orig_run_command(argv, **kwargs)

    _bass_utils.run_command = _run_command_ldwopt
    _bass_utils._ldwopt_patched = True

F32 = mybir.dt.float32
BF16 = mybir.dt.bfloat16
AF = mybir.ActivationFunctionType
OP = mybir.AluOpType

B, S, D, H, K = 8, 1024, 1024, 16, 64
HK = H * K
P = 128
SC = S // P      # 8 s-chunks
DC = D // P      # 8 d-chunks
HKC = HK // P    # 8 hk-chunks (head pairs)
NQW = S // 512   # 2 q-windows of 512
NEG = -1e9

TRACE = False  # set by test.py for profiling runs

_nc_cache = None


def _build_nc(repeat=1):
    nc = bacc.Bacc(None, target_bir_lowering=False)
    if LDW_OPT:
        # leave waits on the matmuls; walrus ldw-opt refuses LDWEIGHTS
        # that carry semaphore waits
        nc.move_matmul_waits_to_ldweights = lambda: None

    x_d = nc.dram_tensor("x", [S, D], BF16, kind="ExternalInput")
    wq_d = nc.dram_tensor("wq", [D, HK], BF16, kind="ExternalInput")
    wk_d = nc.dram_tensor("wk", [D, HK], BF16, kind="ExternalInput")
    wv_d = nc.dram_tensor("wv", [D, HK], BF16, kind="ExternalInput")
    wo_d = nc.dram_tensor("wo", [HK, D], BF16, kind="ExternalInput")
    bq_d = nc.dram_tensor("bq", [HK], F32, kind="ExternalInput")
    bk_d = nc.dram_tensor("bk", [HK], F32, kind="ExternalInput")
    bv_d = nc.dram_tensor("bv", [HK], F32, kind="ExternalInput")
    bo_d = nc.dram_tensor("bo", [D], F32, kind="ExternalInput")
    ka_d = nc.dram_tensor("ka", [S], F32, kind="ExternalInput")   # (m-1)*1e9
    mq_d = nc.dram_tensor("mq", [S], F32, kind="ExternalInput")   # mask 0/1
    oneb_d = nc.dram_tensor("oneb", [1], BF16, kind="ExternalInput")
    out_d = nc.dram_tensor("out", [S, D], F32, kind="ExternalOutput")

    def bcast_ap(t, counts, step_last=None):
        # DRAM AP broadcasting a small tensor across leading 0-stride dims.
        ap = [[0, c] for c in counts]
        ap.append(step_last if step_last is not None else [1, 1])
        return bass.AP(tensor=t, offset=0, ap=ap)

    with tile.TileContext(nc) as tc:
        with (
            tc.tile_pool(name="consts", bufs=1) as consts,
            tc.tile_pool(name="big", bufs=1) as big,
            tc.tile_pool(name="wpool", bufs=1) as wpool,
            tc.tile_pool(name="epool", bufs=6) as epool,
            tc.tile_pool(name="rb", bufs=2) as rbpool,
            tc.tile_pool(name="rp", bufs=4) as rpool,
            tc.tile_pool(name="cn", bufs=2) as cnpool,
            tc.tile_pool(name="op", bufs=2) as opool,
            tc.tile_pool(name="dram", bufs=1, space="DRAM") as drampool,
            tc.tile_pool(name="pmm", bufs=2, space="PSUM") as pmm,
            tc.tile_pool(name="pscore", bufs=2, space="PSUM") as pscore,
            tc.tile_pool(name="pctx", bufs=2, space="PSUM") as pctx,
        ):
            # ---- constant tiles (DMAs emitted in _emit_body AFTER the
            # x transposes so they don't block the SP queue at t=0) ----
            ka_sb = consts.tile([P, SC], F32)
            mq_sb = consts.tile([P, SC], F32)
            bq_sb = consts.tile([P, HKC], F32)
            bk_sb = consts.tile([P, HKC], F32)
            bv_bc = consts.tile([P, HK], F32)
            bo_bc = consts.tile([P, D], F32)
            ones_col = consts.tile([P, 1], BF16)

            loop_cm = (
                tc.For_i(
                    0,
                    repeat,
                    1,
                    hint_engines=(
                        mybir.EngineType.PE,
                        mybir.EngineType.Activation,
                        mybir.EngineType.DVE,
                        mybir.EngineType.SP,
                        mybir.EngineType.Pool,
                    ),
                )
                if repeat > 1
                else contextlib.nullcontext()
            )
            with loop_cm:
                _emit_body(
                    nc, tc, x_d, wq_d, wk_d, wv_d, wo_d, out_d, bcast_ap,
                    oneb_d, ka_sb, mq_sb, bq_sb, bk_sb, bv_bc, bo_bc,
                    ones_col, consts, big, wpool, epool, rbpool, rpool,
                    cnpool, opool, drampool, pmm, pscore, pctx,
                    ka_d, mq_d, bq_d, bk_d, bv_d, bo_d,
                )

    nc.compile()
    return nc


def _emit_body(
    nc, tc, x_d, wq_d, wk_d, wv_d, wo_d, out_d, bcast_ap, oneb_d,
    ka_sb, mq_sb, bq_sb, bk_sb, bv_bc, bo_bc, ones_col, consts, big,
    wpool, epool, rbpool, rpool, cnpool, opool, drampool, pmm, pscore, pctx,
    ka_d, mq_d, bq_d, bk_d, bv_d, bo_d,
):
    # ---- persistent big tensors (all bf16) ----
    xT = big.tile([P, DC * S], BF16, tag="xT", name="xT").rearrange(
        "p (c s) -> p c s", c=DC
    )
    qT = big.tile([P, HKC * S], BF16, tag="qT", name="qT").rearrange(
        "p (c s) -> p c s", c=HKC
    )
    kT = big.tile([P, HKC * S], BF16, tag="kT", name="kT").rearrange(
        "p (c s) -> p c s", c=HKC
    )
    vext = big.tile([P, SC * H * (K + 1)], BF16, tag="vext", name="vext").rearrange(
        "p (s h k) -> p s h k", s=SC, h=H
    )
    ctxT = big.tile([P, HKC * S], BF16, tag="ctxT", name="ctxT").rearrange(
        "p (c s) -> p c s", c=HKC
    )
    # full-row weight layouts [p = row%128, chunk = row//128, 1024] (2KB lines)
    wqs = wpool.tile([P, DC * HK], BF16, tag="wq", name="wqs").rearrange(
        "p (c m) -> p c m", c=DC
    )
    wks = wpool.tile([P, DC * HK], BF16, tag="wk", name="wks").rearrange(
        "p (c m) -> p c m", c=DC
    )
    wvs = wpool.tile([P, DC * HK], BF16, tag="wv", name="wvs").rearrange(
        "p (c m) -> p c m", c=DC
    )
    wos = wpool.tile([P, HKC * D], BF16, tag="wo", name="wos").rearrange(
        "p (c m) -> p c m", c=HKC
    )

    # ---- phase 0: x -> xT via hardware XBAR DMA transpose (SP queue,
    # first in the queue so the v projection can start ASAP) ----
    for dc2 in range(DC // 2):
        nc.sync.dma_start(
            xT[:, 2 * dc2 : 2 * dc2 + 2, :],
            x_d.ap()[:, dc2 * 256 : (dc2 + 1) * 256],
            transpose=True,
        )
    # weight loads on the ACT hardware DGE queue (parallel with SP)
    nc.scalar.dma_start(wvs[:], wv_d.ap().rearrange("(c p) m -> p c m", p=P))
    nc.scalar.dma_start(wqs[:], wq_d.ap().rearrange("(c p) m -> p c m", p=P))
    nc.scalar.dma_start(wks[:], wk_d.ap().rearrange("(c p) m -> p c m", p=P))
    nc.scalar.dma_start(wos[:], wo_d.ap().rearrange("(c p) m -> p c m", p=P))

    # ones column of vext via Pool-engine memset (a broadcast DMA here
    # generates 16K 2-byte descriptors and stalls the SP queue for >100us)
    nc.gpsimd.memset(
        vext[:, :, :, K : K + 1].rearrange("p a b o -> p (a b) o"), 1.0
    )

    # constants: small DMAs behind the x transposes on the SP queue
    nc.sync.dma_start(bv_bc[:], bcast_ap(bv_d, [P], [1, HK]))
    nc.sync.dma_start(bq_sb[:], bq_d.ap().rearrange("(p c) -> p c", p=P))
    nc.sync.dma_start(bk_sb[:], bk_d.ap().rearrange("(p c) -> p c", p=P))
    nc.sync.dma_start(ka_sb[:], ka_d.ap().rearrange("(p c) -> p c", p=P))
    nc.sync.dma_start(mq_sb[:], mq_d.ap().rearrange("(p c) -> p c", p=P))
    nc.sync.dma_start(bo_bc[:], bcast_ap(bo_d, [P], [1, D]))
    nc.sync.dma_start(ones_col[:], bcast_ap(oneb_d, [P]))

    # ---- phase 1a: v projection into vext; qk chunk-0 projection steps
    # interleaved (the v phase is paced by the x transposes, leaving PE
    # slack the qk0 matmuls can fill) ----
    def v_phase():
        qi = 0
        for hh in range(2):  # hk halves of 512
            for st in range(SC):
                ps = pmm.tile([P, 512], F32, tag="mm", name="ps")
                for dc in range(DC):
                    nc.tensor.matmul(
                        ps[:],
                        xT[:, dc, st * P : (st + 1) * P],
                        wvs[:, dc, hh * 512 : (hh + 1) * 512],
                        start=(dc == 0),
                        stop=(dc == DC - 1),
                    )
                nc.vector.tensor_tensor(
                    vext[:, st, hh * 8 : (hh + 1) * 8, 0:K],
                    ps[:].rearrange("p (h k) -> p h k", k=K),
                    bv_bc[:, hh * 512 : (hh + 1) * 512].rearrange(
                        "p (h k) -> p h k", k=K
                    ),
                    OP.add,
                )
                for _ in range(3):
                    if qi < len(qk0):
                        qk0[qi]()
                        qi += 1
        while qi < len(qk0):
            qk0[qi]()
            qi += 1

    # ---- qk projection steps (emitted interleaved with attention) ----
    # matmul computes lhsT.T @ rhs: for qT [hk, s] use lhsT = W chunk
    # [d, hk-cols], rhs = xT [d, s].
    def proj_chunk_steps(hkc):
        steps = []
        for w_sb, b_sb, dst in ((wqs, bq_sb, qT), (wks, bk_sb, kT)):
            for qw in range(NQW):
                ps_box = []

                def alloc(ps_box=ps_box):
                    ps_box.append(pmm.tile([P, 512], F32, tag="mm", name="ps"))

                steps.append(alloc)
                for dc in range(DC):
                    def mm(dc=dc, w_sb=w_sb, qw=qw, hkc=hkc, ps_box=ps_box):
                        nc.tensor.matmul(
                            ps_box[0][:],
                            w_sb[:, dc, hkc * P : (hkc + 1) * P],
                            xT[:, dc, qw * 512 : (qw + 1) * 512],
                            start=(dc == 0),
                            stop=(dc == DC - 1),
                        )
                    steps.append(mm)

                def bias(b_sb=b_sb, dst=dst, qw=qw, hkc=hkc, ps_box=ps_box):
                    nc.vector.tensor_scalar_add(
                        dst[:, hkc, qw * 512 : (qw + 1) * 512],
                        ps_box[0][:],
                        b_sb[:, hkc : hkc + 1],
                    )
                steps.append(bias)
        return steps

    # ---- u-path steps (uniform-attention fixup), emitted during chunk 7 ----
    mvh = consts.tile([1, HK], BF16)
    mvT = consts.tile([P, HKC], BF16)
    u_bc = consts.tile([P, D], F32)

    def upath_steps():
        steps = []
        for hh in range(2):
            ps_box = []

            def alloc(ps_box=ps_box):
                ps_box.append(pmm.tile([P, 512], F32, tag="mm", name="ps"))

            steps.append(alloc)
            for sc in range(SC):
                def mm(sc=sc, hh=hh, ps_box=ps_box):
                    nc.tensor.matmul(
                        ps_box[0][0:1, :].rearrange("o (h k) -> o h k", k=K),
                        ones_col[:],
                        vext[:, sc, hh * 8 : (hh + 1) * 8, 0:K],
                        start=(sc == 0),
                        stop=(sc == SC - 1),
                    )
                steps.append(mm)

            def fin(hh=hh, ps_box=ps_box):
                nc.vector.tensor_scalar_mul(
                    mvh[0:1, hh * 512 : (hh + 1) * 512],
                    ps_box[0][0:1, :],
                    1.0 / S,
                )
            steps.append(fin)

        # transpose mvh [1, HK] -> mvT [128, HKC] via 8 tiny N=1 matmuls
        # (avoids a DRAM roundtrip + a 1024x2B-descriptor DMA)
        def loadmvT():
            pmv = pmm.tile([P, 512], F32, tag="mm", name="ps")
            for c in range(HKC):
                nc.tensor.matmul(
                    pmv[:, c : c + 1],
                    mvh[0:1, c * P : (c + 1) * P],
                    ones_col[0:1, :],
                    start=True,
                    stop=True,
                )
            nc.vector.tensor_copy(mvT[:], pmv[:, 0:HKC])
        steps.append(loadmvT)

        for dh in range(2):
            ps_box = []

            def alloc(ps_box=ps_box):
                ps_box.append(pmm.tile([P, 512], F32, tag="mm", name="ps"))

            steps.append(alloc)
            for c in range(HKC):
                def mm(c=c, dh=dh, ps_box=ps_box):
                    nc.tensor.matmul(
                        ps_box[0][0:1, :],
                        mvT[:, c : c + 1],
                        wos[:, c, dh * 512 : (dh + 1) * 512],
                        start=(c == 0),
                        stop=(c == HKC - 1),
                    )
                steps.append(mm)

            def fin(dh=dh, ps_box=ps_box):
                uh = rpool.tile([1, 512], F32, tag="rp", name="uh")
                nc.vector.tensor_copy(uh[:], ps_box[0][0:1, :])
                nc.gpsimd.partition_broadcast(
                    u_bc[:, dh * 512 : (dh + 1) * 512], uh[:]
                )
            steps.append(fin)

        def addbo():
            nc.vector.tensor_tensor(u_bc[:], u_bc[:], bo_bc[:], OP.add)
        steps.append(addbo)
        return steps

    # ---- output-projection step for one (qt, dh): 8 matmuls + blend ----
    # qt < 4 reads only the qw0 half of ctxT (query rows < 512), so those
    # chunks can interleave into chunk 7's qw1 attention — keeping the PE
    # busy across the attention->projection transition (otherwise a ~6us
    # PE gap lets the HAM clock-gate re-throttle to 1.2 GHz for the tail).
    def outproj_step(qt, dh):
        def f():
            po = pmm.tile([P, 512], F32, tag="mm", name="ps")
            for c in range(HKC):
                nc.tensor.matmul(
                    po[:],
                    ctxT[:, c, qt * P : (qt + 1) * P],
                    wos[:, c, dh * 512 : (dh + 1) * 512],
                    start=(c == 0),
                    stop=(c == HKC - 1),
                )
            # out = (po - (u+bo))*mq + (u+bo)
            ub = u_bc[:, dh * 512 : (dh + 1) * 512]
            t1 = opool.tile([P, 512], F32, tag="o1", name="t1")
            nc.vector.tensor_tensor(t1[:], po[:], ub, OP.subtract)
            nc.vector.scalar_tensor_tensor(
                t1[:], t1[:], mq_sb[:, qt : qt + 1], ub, OP.mult, OP.add
            )
            nc.sync.dma_start(
                out_d.ap()[
                    qt * P : (qt + 1) * P, dh * 512 : (dh + 1) * 512
                ],
                t1[:],
            )
        return f

    # ---- phase 1a+1b: v projection with qk chunk 0 interleaved ----
    qk0 = proj_chunk_steps(0)
    v_phase()

    # ---- phase 2: attention per head pair, proj chunk hc+1 interleaved ----
    for hc in range(HKC):
        if hc + 1 < HKC:
            steps_all = proj_chunk_steps(hc + 1)
            half = (len(steps_all) + 1) // 2
            pending_by_qw = [steps_all[:half], steps_all[half:]]
        else:
            pending_by_qw = [
                upath_steps(),
                [outproj_step(qt, dh) for qt in range(4) for dh in range(2)],
            ]

        hA, hB = 2 * hc, 2 * hc + 1
        for qw in range(NQW):
            pending = pending_by_qw[qw]
            nslots = SC + 1
            per_slot = (len(pending) + nslots - 1) // nslots
            pi = 0
            pcA = pctx.tile([P, 512], F32, tag="ctx", name="pcA")
            pcB = pctx.tile([P, 512], F32, tag="ctx", name="pcB")
            exs = {}
            for sc in range(SC + 1):
                if sc < SC:
                    pss2 = pscore.tile([P, 1024], F32, tag="score", name="pss2")
                    nc.tensor.matmul(
                        pss2[:, 0:512],
                        kT[0:64, hc, sc * P : (sc + 1) * P],
                        qT[0:64, hc, qw * 512 : (qw + 1) * 512],
                        start=True,
                        stop=True,
                    )
                    nc.tensor.matmul(
                        pss2[:, 512:1024],
                        kT[64:128, hc, sc * P : (sc + 1) * P],
                        qT[64:128, hc, qw * 512 : (qw + 1) * 512],
                        start=True,
                        stop=True,
                    )
                    ex = epool.tile([P, 1024], BF16, tag="exp", name="ex")
                    nc.scalar.activation(
                        ex[:], pss2[:], AF.Exp,
                        bias=ka_sb[:, sc : sc + 1], scale=1.0,
                    )
                    exs[sc] = ex
                # interleaved proj/upath steps (cover ACT latency)
                for _ in range(per_slot):
                    if pi < len(pending):
                        pending[pi]()
                        pi += 1
                if sc >= 1:
                    exm = exs.pop(sc - 1)
                    nc.tensor.matmul(
                        pcA[0:65, :],
                        vext[:, sc - 1, hA, :],
                        exm[:, 0:512],
                        start=(sc - 1 == 0),
                        stop=(sc - 1 == SC - 1),
                    )
                    nc.tensor.matmul(
                        pcB[0:65, :],
                        vext[:, sc - 1, hB, :],
                        exm[:, 512:1024],
                        start=(sc - 1 == 0),
                        stop=(sc - 1 == SC - 1),
                    )
            while pi < len(pending):
                pending[pi]()
                pi += 1
            # ---- normalization (off the PSUM critical path): copy each
            # [65,512] accumulator to SBUF first — frees the PSUM bank for
            # the next q-window earlier, and feeds reciprocal_approx_fast
            # from SBUF (from PSUM the bit-trick seed reads garbage on HW).
            pcsA = cnpool.tile([64, 512], F32, tag="pcs", name="pcsA")
            nc.vector.tensor_copy(pcsA[:], pcA[0:64, :])
            sumA = rpool.tile([1, 512], F32, tag="rp", name="sumA")
            nc.vector.tensor_copy(sumA[:], pcA[64:65, :])
            recipA = rpool.tile([1, 512], F32, tag="rp", name="recipA")
            nc.vector.reciprocal_approx_fast(recipA[:], sumA[:])
            rbA = rbpool.tile([64, 512], F32, tag="rb", name="rbA")
            nc.gpsimd.partition_broadcast(rbA[:], recipA[:])
            nc.vector.tensor_tensor(
                ctxT[0:64, hc, qw * 512 : (qw + 1) * 512],
                pcsA[:],
                rbA[:],
                OP.mult,
            )
            # ---- odd head: scratch + partition-shift DMA ----
            pcsB = cnpool.tile([64, 512], F32, tag="pcs", name="pcsB")
            nc.vector.tensor_copy(pcsB[:], pcB[0:64, :])
            sumB = rpool.tile([1, 512], F32, tag="rp", name="sumB")
            nc.vector.tensor_copy(sumB[:], pcB[64:65, :])
            recipB = rpool.tile([1, 512], F32, tag="rp", name="recipB")
            nc.vector.reciprocal_approx_fast(recipB[:], sumB[:])
            rbB = rbpool.tile([64, 512], F32, tag="rb", name="rbB")
            nc.gpsimd.partition_broadcast(rbB[:], recipB[:])
            cnB = cnpool.tile([64, 512], BF16, tag="cn", name="cnB")
            nc.vector.tensor_tensor(cnB[:], pcsB[:], rbB[:], OP.mult)
            nc.sync.dma_start(
                ctxT[64:128, hc, qw * 512 : (qw + 1) * 512], cnB[:]
            )

    # ---- phase 3: remaining output projection (qt 4-7 need qw1 ctxT) ----
    for qt in range(4, SC):
        for dh in range(2):
            outproj_step(qt, dh)()


def _get_nc():
    global _nc_cache
    if _nc_cache is None:
        _nc_cache = _build_nc()
    return _nc_cache


_nc_bench_cache = {}


def _get_bench_nc(repeat):
    if repeat not in _nc_bench_cache:
        _nc_bench_cache[repeat] = _build_nc(repeat)
    return _nc_bench_cache[repeat]


def _prep_in_maps(input_tensor, input_mask, Wq, bq, Wk, bk, Wv, bv, Wo, bo):
    bf16 = ml_dtypes.bfloat16
    x = np.ascontiguousarray(np.asarray(input_tensor, dtype=np.float32))
    mask = np.asarray(input_mask).astype(bool)
    Wq = np.asarray(Wq, dtype=np.float32).reshape(D, HK)
    Wk = np.asarray(Wk, dtype=np.float32).reshape(D, HK)
    Wv = np.asarray(Wv, dtype=np.float32).reshape(D, HK)
    Wo = np.asarray(Wo, dtype=np.float32).reshape(HK, D)
    bq = np.asarray(bq, dtype=np.float32).reshape(HK)
    bk = np.asarray(bk, dtype=np.float32).reshape(HK)
    bv = np.asarray(bv, dtype=np.float32).reshape(HK)
    bo = np.asarray(bo, dtype=np.float32).reshape(D)

    # fold the 1/sqrt(K)=1/8 score scale into Wq/bq (exact: power of two)
    wqs = np.ascontiguousarray((Wq / 8.0).astype(bf16))
    bqs = bq / 8.0
    wkb = np.ascontiguousarray(Wk.astype(bf16))
    wvb = np.ascontiguousarray(Wv.astype(bf16))
    wob = np.ascontiguousarray(Wo.astype(bf16))

    mf = mask.astype(np.float32)
    ka = (mf - 1.0) * 1e9   # 0 where kept, -1e9 where masked
    oneb = np.ones(1, bf16)

    def perm(v):
        # [n*128] chunk-major -> per-partition-contiguous [(p c)] layout
        return np.ascontiguousarray(v.reshape(-1, P).T).reshape(-1)

    bqp = perm(bqs)
    bkp = perm(bk)

    xb = x.astype(bf16)

    in_maps = []
    for b in range(B):
        in_maps.append(
            {
                "x": np.ascontiguousarray(xb[b]),
                "wq": wqs,
                "wk": wkb,
                "wv": wvb,
                "wo": wob,
                "bq": bqp,
                "bk": bkp,
                "bv": np.ascontiguousarray(bv),
                "bo": np.ascontiguousarray(bo),
                "ka": perm(ka[b]),
                "mq": perm(mf[b]),
                "oneb": oneb,
            }
        )
    return in_maps


def kernel(input_tensor, input_mask, Wq, bq, Wk, bk, Wv, bv, Wo, bo):
    in_maps = _prep_in_maps(
        input_tensor, input_mask, Wq, bq, Wk, bk, Wv, bv, Wo, bo
    )
    nc = _get_nc()
    res = run_bass_kernel_spmd(nc, in_maps, core_ids=list(range(B)), trace=TRACE)
    if TRACE:
        kernel.last_result = res
    out = np.stack([r["out"] for r in res.results], axis=0)
    return out


# revision 33
# speedup vs baseline: 1.1640x; 1.1640x over previous
"""Trainium2 Bass kernel for CustomSelfAttention (B=8,S=1024,D=1024,H=16,K=64).

Strategy: data-parallel over batch across 8 NeuronCores (1 batch item/core).
All matmuls in bf16 (host pre-casts x and weights; 1 cycle/column on the PE
vs 2 for fp32, FWL weight loads). Per-core pipeline:
  0. xT [d, s] via hardware XBAR DMA-transpose of bf16 x (no PE transposes).
  1. qT = (Wq/8)^T x^T, kT = Wk^T x^T (layout [hk, s]); v = x Wv stored
     interleaved with a ones column per head: vext [s, h, 65].
  2. attention per head PAIR (even head on PE rows 0-63, odd on 64-127 via
     tile_position row tiling -> the two K=64 scores matmuls run
     concurrently); one Exp activation over a 2-bank PSUM tile [128,1024]
     with the per-partition key-mask bias fused; ctx matmuls with
     lhsT=[v_h | 1] give ctx^T[k,q] plus softmax row sums in one shot;
     normalize with reciprocal_approx_fast + partition_broadcast.
     QKV projection matmuls for chunk c+1 are interleaved into attention
     chunk c's PE queue to cover the ACT-bound exp latency.
  3. out = ctxT^T Wo + bo, blended with the uniform-attention row for
     fully-masked queries (softmax of a row of -1e9 is exactly uniform),
     computed as u = mean_s(v) Wo.
"""

import contextlib
import sys
import types

sys.path.insert(0, "/opt/trn_rl_repo")

# The image's antenv package may lack axon_hooks (NTFF profile hook
# registry); bass_utils imports it unconditionally when trace=True.
# Install a functional shim + register the ctypes hook like
# trn_agent_boot.trn_boot does.
try:
    import antenv.axon_hooks  # noqa: F401
except ImportError:
    try:
        import antenv

        _hooks_mod = types.ModuleType("antenv.axon_hooks")
        _hook_box = [None]
        _hooks_mod.get_axon_ntff_profile_hook = lambda: _hook_box[0]
        _hooks_mod.set_axon_ntff_profile_hook = (
            lambda h: _hook_box.__setitem__(0, h)
        )
        sys.modules["antenv.axon_hooks"] = _hooks_mod
        antenv.axon_hooks = _hooks_mod
        from trn_agent_boot.trn_boot import _ntff_profile_via_ctypes

        _hooks_mod.set_axon_ntff_profile_hook(
            _ntff_profile_via_ctypes("/opt/axon/libaxon_pjrt.so")
        )
    except Exception:
        pass

import ml_dtypes  # noqa: E402
import numpy as np  # noqa: E402

import concourse.bass as bass  # noqa: E402
import concourse.bass_utils as _bass_utils  # noqa: E402
import concourse.mybir as mybir  # noqa: E402
import concourse.tile as tile  # noqa: E402
from concourse import bacc  # noqa: E402
from concourse.bass_utils import run_bass_kernel_spmd  # noqa: E402
from concourse.masks import make_identity  # noqa: E402

# Enable the walrus LDWEIGHTS background-buffer optimization for this
# kernel's compile: without it every MATMUL serializes behind its
# foreground weight load (~+170ns per matmul on this kernel). Walrus
# rejects ldw-opt when LDWEIGHTS carry semaphore waits, so the bass pass
# that moves matmul waits onto LDWEIGHTS must be skipped too (see
# _build_nc).
LDW_OPT = False

if not getattr(_bass_utils, "_ldwopt_patched", False):
    _orig_run_command = _bass_utils.run_command

    def _run_command_ldwopt(argv, **kwargs):
        if LDW_OPT and isinstance(argv, list):
            argv = [
                "--enable-ldw-opt=true" if a == "--enable-ldw-opt=false" else a
                for a in argv
            ]
        return _orig_run_command(argv, **kwargs)

    _bass_utils.run_command = _run_command_ldwopt
    _bass_utils._ldwopt_patched = True

F32 = mybir.dt.float32
BF16 = mybir.dt.bfloat16
AF = mybir.ActivationFunctionType
OP = mybir.AluOpType

B, S, D, H, K = 8, 1024, 1024, 16, 64
HK = H * K
P = 128
SC = S // P      # 8 s-chunks
DC = D // P      # 8 d-chunks
HKC = HK // P    # 8 hk-chunks (head pairs)
NQW = S // 512   # 2 q-windows of 512
NEG = -1e9

TRACE = False  # set by test.py for profiling runs

_nc_cache = None


def _build_nc(repeat=1):
    nc = bacc.Bacc(None, target_bir_lowering=False)
    if LDW_OPT:
        # leave waits on the matmuls; walrus ldw-opt refuses LDWEIGHTS
        # that carry semaphore waits
        nc.move_matmul_waits_to_ldweights = lambda: None

    x_d = nc.dram_tensor("x", [S, D], BF16, kind="ExternalInput")
    wq_d = nc.dram_tensor("wq", [D, HK], BF16, kind="ExternalInput")
    wk_d = nc.dram_tensor("wk", [D, HK], BF16, kind="ExternalInput")
    wv_d = nc.dram_tensor("wv", [D, HK], BF16, kind="ExternalInput")
    wo_d = nc.dram_tensor("wo", [HK, D], BF16, kind="ExternalInput")
    bq_d = nc.dram_tensor("bq", [HK], F32, kind="ExternalInput")
    bk_d = nc.dram_tensor("bk", [HK], F32, kind="ExternalInput")
    bv_d = nc.dram_tensor("bv", [HK], F32, kind="ExternalInput")
    bo_d = nc.dram_tensor("bo", [D], F32, kind="ExternalInput")
    ka_d = nc.dram_tensor("ka", [S], F32, kind="ExternalInput")   # (m-1)*1e9
    mq_d = nc.dram_tensor("mq", [S], F32, kind="ExternalInput")   # mask 0/1
    oneb_d = nc.dram_tensor("oneb", [1], BF16, kind="ExternalInput")
    out_d = nc.dram_tensor("out", [S, D], F32, kind="ExternalOutput")

    def bcast_ap(t, counts, step_last=None):
        # DRAM AP broadcasting a small tensor across leading 0-stride dims.
        ap = [[0, c] for c in counts]
        ap.append(step_last if step_last is not None else [1, 1])
        return bass.AP(tensor=t, offset=0, ap=ap)

    with tile.TileContext(nc) as tc:
        with (
            tc.tile_pool(name="consts", bufs=1) as consts,
            tc.tile_pool(name="big", bufs=1) as big,
            tc.tile_pool(name="wpool", bufs=1) as wpool,
            tc.tile_pool(name="epool", bufs=6) as epool,
            tc.tile_pool(name="rb", bufs=2) as rbpool,
            tc.tile_pool(name="rp", bufs=4) as rpool,
            tc.tile_pool(name="cn", bufs=2) as cnpool,
            tc.tile_pool(name="op", bufs=2) as opool,
            tc.tile_pool(name="dram", bufs=1, space="DRAM") as drampool,
            tc.tile_pool(name="pmm", bufs=2, space="PSUM") as pmm,
            tc.tile_pool(name="pscore", bufs=2, space="PSUM") as pscore,
            tc.tile_pool(name="pctx", bufs=2, space="PSUM") as pctx,
        ):
            # ---- constant tiles (DMAs emitted in _emit_body AFTER the
            # x transposes so they don't block the SP queue at t=0) ----
            ka_sb = consts.tile([P, SC], F32)
            mq_sb = consts.tile([P, SC], F32)
            bq_sb = consts.tile([P, HKC], F32)
            bk_sb = consts.tile([P, HKC], F32)
            bv_bc = consts.tile([P, HK], F32)
            bo_bc = consts.tile([P, D], F32)
            ones_col = consts.tile([P, 1], BF16)

            loop_cm = (
                tc.For_i(
                    0,
                    repeat,
                    1,
                    hint_engines=(
                        mybir.EngineType.PE,
                        mybir.EngineType.Activation,
                        mybir.EngineType.DVE,
                        mybir.EngineType.SP,
                        mybir.EngineType.Pool,
                    ),
                )
                if repeat > 1
                else contextlib.nullcontext()
            )
            with loop_cm:
                _emit_body(
                    nc, tc, x_d, wq_d, wk_d, wv_d, wo_d, out_d, bcast_ap,
                    oneb_d, ka_sb, mq_sb, bq_sb, bk_sb, bv_bc, bo_bc,
                    ones_col, consts, big, wpool, epool, rbpool, rpool,
                    cnpool, opool, drampool, pmm, pscore, pctx,
                    ka_d, mq_d, bq_d, bk_d, bv_d, bo_d,
                )

    nc.compile()
    return nc


def _emit_body(
    nc, tc, x_d, wq_d, wk_d, wv_d, wo_d, out_d, bcast_ap, oneb_d,
    ka_sb, mq_sb, bq_sb, bk_sb, bv_bc, bo_bc, ones_col, consts, big,
    wpool, epool, rbpool, rpool, cnpool, opool, drampool, pmm, pscore, pctx,
    ka_d, mq_d, bq_d, bk_d, bv_d, bo_d,
):
    # ---- persistent big tensors (all bf16) ----
    xT = big.tile([P, DC * S], BF16, tag="xT", name="xT").rearrange(
        "p (c s) -> p c s", c=DC
    )
    qT = big.tile([P, HKC * S], BF16, tag="qT", name="qT").rearrange(
        "p (c s) -> p c s", c=HKC
    )
    kT = big.tile([P, HKC * S], BF16, tag="kT", name="kT").rearrange(
        "p (c s) -> p c s", c=HKC
    )
    vext = big.tile([P, SC * H * (K + 1)], BF16, tag="vext", name="vext").rearrange(
        "p (s h k) -> p s h k", s=SC, h=H
    )
    ctxT = big.tile([P, HKC * S], BF16, tag="ctxT", name="ctxT").rearrange(
        "p (c s) -> p c s", c=HKC
    )
    # full-row weight layouts [p = row%128, chunk = row//128, 1024] (2KB lines)
    wqs = wpool.tile([P, DC * HK], BF16, tag="wq", name="wqs").rearrange(
        "p (c m) -> p c m", c=DC
    )
    wks = wpool.tile([P, DC * HK], BF16, tag="wk", name="wks").rearrange(
        "p (c m) -> p c m", c=DC
    )
    wvs = wpool.tile([P, DC * HK], BF16, tag="wv", name="wvs").rearrange(
        "p (c m) -> p c m", c=DC
    )
    wos = wpool.tile([P, HKC * D], BF16, tag="wo", name="wos").rearrange(
        "p (c m) -> p c m", c=HKC
    )

    # ---- phase 0: x -> xT via hardware XBAR DMA transpose (SP queue,
    # first in the queue so the v projection can start ASAP) ----
    for dc2 in range(DC // 2):
        nc.sync.dma_start(
            xT[:, 2 * dc2 : 2 * dc2 + 2, :],
            x_d.ap()[:, dc2 * 256 : (dc2 + 1) * 256],
            transpose=True,
        )
    # weight loads on the ACT hardware DGE queue (parallel with SP)
    nc.scalar.dma_start(wvs[:], wv_d.ap().rearrange("(c p) m -> p c m", p=P))
    nc.scalar.dma_start(wqs[:], wq_d.ap().rearrange("(c p) m -> p c m", p=P))
    nc.scalar.dma_start(wks[:], wk_d.ap().rearrange("(c p) m -> p c m", p=P))
    nc.scalar.dma_start(wos[:], wo_d.ap().rearrange("(c p) m -> p c m", p=P))

    # ones column of vext via Pool-engine memset (a broadcast DMA here
    # generates 16K 2-byte descriptors and stalls the SP queue for >100us)
    nc.gpsimd.memset(
        vext[:, :, :, K : K + 1].rearrange("p a b o -> p (a b) o"), 1.0
    )

    # constants: small DMAs behind the x transposes on the SP queue
    nc.sync.dma_start(bv_bc[:], bcast_ap(bv_d, [P], [1, HK]))
    nc.sync.dma_start(bq_sb[:], bq_d.ap().rearrange("(p c) -> p c", p=P))
    nc.sync.dma_start(bk_sb[:], bk_d.ap().rearrange("(p c) -> p c", p=P))
    nc.sync.dma_start(ka_sb[:], ka_d.ap().rearrange("(p c) -> p c", p=P))
    nc.sync.dma_start(mq_sb[:], mq_d.ap().rearrange("(p c) -> p c", p=P))
    nc.sync.dma_start(bo_bc[:], bcast_ap(bo_d, [P], [1, D]))
    nc.sync.dma_start(ones_col[:], bcast_ap(oneb_d, [P]))

    # ---- phase 1a: v projection into vext; qk chunk-0 projection steps
    # interleaved (the v phase is paced by the x transposes, leaving PE
    # slack the qk0 matmuls can fill) ----
    def v_phase():
        qi = 0
        for hh in range(2):  # hk halves of 512
            for st in range(SC):
                ps = pmm.tile([P, 512], F32, tag="mm", name="ps")
                for dc in range(DC):
                    nc.tensor.matmul(
                        ps[:],
                        xT[:, dc, st * P : (st + 1) * P],
                        wvs[:, dc, hh * 512 : (hh + 1) * 512],
                        start=(dc == 0),
                        stop=(dc == DC - 1),
                    )
                nc.vector.tensor_tensor(
                    vext[:, st, hh * 8 : (hh + 1) * 8, 0:K],
                    ps[:].rearrange("p (h k) -> p h k", k=K),
                    bv_bc[:, hh * 512 : (hh + 1) * 512].rearrange(
                        "p (h k) -> p h k", k=K
                    ),
                    OP.add,
                )
                for _ in range(3):
                    if qi < len(qk0):
                        qk0[qi]()
                        qi += 1
        while qi < len(qk0):
            qk0[qi]()
            qi += 1

    # ---- qk projection steps (emitted interleaved with attention) ----
    # matmul computes lhsT.T @ rhs: for qT [hk, s] use lhsT = W chunk
    # [d, hk-cols], rhs = xT [d, s].
    def proj_chunk_steps(hkc):
        steps = []
        for w_sb, b_sb, dst in ((wqs, bq_sb, qT), (wks, bk_sb, kT)):
            for qw in range(NQW):
                ps_box = []

                def alloc(ps_box=ps_box):
                    ps_box.append(pmm.tile([P, 512], F32, tag="mm", name="ps"))

                steps.append(alloc)
                for dc in range(DC):
                    def mm(dc=dc, w_sb=w_sb, qw=qw, hkc=hkc, ps_box=ps_box):
                        nc.tensor.matmul(
                            ps_box[0][:],
                            w_sb[:, dc, hkc * P : (hkc + 1) * P],
                            xT[:, dc, qw * 512 : (qw + 1) * 512],
                            start=(dc == 0),
                            stop=(dc == DC - 1),
                        )
                    steps.append(mm)

                def bias(b_sb=b_sb, dst=dst, qw=qw, hkc=hkc, ps_box=ps_box):
                    nc.vector.tensor_scalar_add(
                        dst[:, hkc, qw * 512 : (qw + 1) * 512],
                        ps_box[0][:],
                        b_sb[:, hkc : hkc + 1],
                    )
                steps.append(bias)
        return steps

    # ---- u-path steps (uniform-attention fixup), emitted during chunk 7 ----
    mvh = consts.tile([1, HK], BF16)
    mvT = consts.tile([P, HKC], BF16)
    u_bc = consts.tile([P, D], F32)

    def upath_steps():
        steps = []
        for hh in range(2):
            ps_box = []

            def alloc(ps_box=ps_box):
                ps_box.append(pmm.tile([P, 512], F32, tag="mm", name="ps"))

            steps.append(alloc)
            for sc in range(SC):
                def mm(sc=sc, hh=hh, ps_box=ps_box):
                    nc.tensor.matmul(
                        ps_box[0][0:1, :].rearrange("o (h k) -> o h k", k=K),
                        ones_col[:],
                        vext[:, sc, hh * 8 : (hh + 1) * 8, 0:K],
                        start=(sc == 0),
                        stop=(sc == SC - 1),
                    )
                steps.append(mm)

            def fin(hh=hh, ps_box=ps_box):
                nc.vector.tensor_scalar_mul(
                    mvh[0:1, hh * 512 : (hh + 1) * 512],
                    ps_box[0][0:1, :],
                    1.0 / S,
                )
            steps.append(fin)

        # transpose mvh [1, HK] -> mvT [128, HKC] via 8 tiny N=1 matmuls
        # (avoids a DRAM roundtrip + a 1024x2B-descriptor DMA)
        def loadmvT():
            pmv = pmm.tile([P, 512], F32, tag="mm", name="ps")
            for c in range(HKC):
                nc.tensor.matmul(
                    pmv[:, c : c + 1],
                    mvh[0:1, c * P : (c + 1) * P],
                    ones_col[0:1, :],
                    start=True,
                    stop=True,
                )
            nc.vector.tensor_copy(mvT[:], pmv[:, 0:HKC])
        steps.append(loadmvT)

        for dh in range(2):
            ps_box = []

            def alloc(ps_box=ps_box):
                ps_box.append(pmm.tile([P, 512], F32, tag="mm", name="ps"))

            steps.append(alloc)
            for c in range(HKC):
                def mm(c=c, dh=dh, ps_box=ps_box):
                    nc.tensor.matmul(
                        ps_box[0][0:1, :],
                        mvT[:, c : c + 1],
                        wos[:, c, dh * 512 : (dh + 1) * 512],
                        start=(c == 0),
                        stop=(c == HKC - 1),
                    )
                steps.append(mm)

            def fin(dh=dh, ps_box=ps_box):
                uh = rpool.tile([1, 512], F32, tag="rp", name="uh")
                nc.vector.tensor_copy(uh[:], ps_box[0][0:1, :])
                nc.gpsimd.partition_broadcast(
                    u_bc[:, dh * 512 : (dh + 1) * 512], uh[:]
                )
            steps.append(fin)

        def addbo():
            nc.vector.tensor_tensor(u_bc[:], u_bc[:], bo_bc[:], OP.add)
        steps.append(addbo)
        return steps

    # ---- output-projection step for one (qt, dh): 8 matmuls + blend ----
    # qt < 4 reads only the qw0 half of ctxT (query rows < 512), so those
    # chunks can interleave into chunk 7's qw1 attention — keeping the PE
    # busy across the attention->projection transition (otherwise a ~6us
    # PE gap lets the HAM clock-gate re-throttle to 1.2 GHz for the tail).
    def outproj_step(qt, dh):
        def f():
            po = pmm.tile([P, 512], F32, tag="mm", name="ps")
            for c in range(HKC):
                nc.tensor.matmul(
                    po[:],
                    ctxT[:, c, qt * P : (qt + 1) * P],
                    wos[:, c, dh * 512 : (dh + 1) * 512],
                    start=(c == 0),
                    stop=(c == HKC - 1),
                )
            # out = (po - (u+bo))*mq + (u+bo)
            ub = u_bc[:, dh * 512 : (dh + 1) * 512]
            t1 = opool.tile([P, 512], F32, tag="o1", name="t1")
            nc.vector.tensor_tensor(t1[:], po[:], ub, OP.subtract)
            nc.vector.scalar_tensor_tensor(
                t1[:], t1[:], mq_sb[:, qt : qt + 1], ub, OP.mult, OP.add
            )
            nc.sync.dma_start(
                out_d.ap()[
                    qt * P : (qt + 1) * P, dh * 512 : (dh + 1) * 512
                ],
                t1[:],
            )
        return f

    # ---- phase 1a+1b: v projection with qk chunk 0 interleaved ----
    qk0 = proj_chunk_steps(0)
    v_phase()

    # ---- phase 2: attention per head pair, proj chunk hc+1 interleaved ----
    for hc in range(HKC):
        if hc + 1 < HKC:
            steps_all = proj_chunk_steps(hc + 1)
            half = (len(steps_all) + 1) // 2
            pending_by_qw = [steps_all[:half], steps_all[half:]]
        else:
            pending_by_qw = [
                upath_steps(),
                [outproj_step(qt, dh) for qt in range(4) for dh in range(2)],
            ]

        hA, hB = 2 * hc, 2 * hc + 1
        for qw in range(NQW):
            pending = pending_by_qw[qw]
            nslots = SC + 1
            per_slot = (len(pending) + nslots - 1) // nslots
            pi = 0
            pcA = pctx.tile([P, 512], F32, tag="ctx", name="pcA")
            pcB = pctx.tile([P, 512], F32, tag="ctx", name="pcB")
            exs = {}
            for sc in range(SC + 1):
                if sc < SC:
                    pss2 = pscore.tile([P, 1024], F32, tag="score", name="pss2")
                    nc.tensor.matmul(
                        pss2[:, 0:512],
                        kT[0:64, hc, sc * P : (sc + 1) * P],
                        qT[0:64, hc, qw * 512 : (qw + 1) * 512],
                        start=True,
                        stop=True,
                    )
                    nc.tensor.matmul(
                        pss2[:, 512:1024],
                        kT[64:128, hc, sc * P : (sc + 1) * P],
                        qT[64:128, hc, qw * 512 : (qw + 1) * 512],
                        start=True,
                        stop=True,
                    )
                    ex = epool.tile([P, 1024], BF16, tag="exp", name="ex")
                    nc.scalar.activation(
                        ex[:], pss2[:], AF.Exp,
                        bias=ka_sb[:, sc : sc + 1], scale=1.0,
                    )
                    exs[sc] = ex
                # interleaved proj/upath steps (cover ACT latency)
                for _ in range(per_slot):
                    if pi < len(pending):
                        pending[pi]()
                        pi += 1
                if sc >= 1:
                    exm = exs.pop(sc - 1)
                    nc.tensor.matmul(
                        pcA[0:65, :],
                        vext[:, sc - 1, hA, :],
                        exm[:, 0:512],
                        start=(sc - 1 == 0),
                        stop=(sc - 1 == SC - 1),
                    )
                    nc.tensor.matmul(
                        pcB[0:65, :],
                        vext[:, sc - 1, hB, :],
                        exm[:, 512:1024],
                        start=(sc - 1 == 0),
                        stop=(sc - 1 == SC - 1),
                    )
            while pi < len(pending):
                pending[pi]()
                pi += 1
            # ---- normalization (off the PSUM critical path): copy each
            # [65,512] accumulator to SBUF first — frees the PSUM bank for
            # the next q-window earlier, and feeds reciprocal_approx_fast
            # from SBUF (from PSUM the bit-trick seed reads garbage on HW).
            pcsA = cnpool.tile([64, 512], F32, tag="pcs", name="pcsA")
            nc.vector.tensor_copy(pcsA[:], pcA[0:64, :])
            sumA = rpool.tile([1, 512], F32, tag="rp", name="sumA")
            nc.vector.tensor_copy(sumA[:], pcA[64:65, :])
            recipA = rpool.tile([1, 512], F32, tag="rp", name="recipA")
            nc.vector.reciprocal_approx_fast(recipA[:], sumA[:])
            rbA = rbpool.tile([64, 512], F32, tag="rb", name="rbA")
            nc.gpsimd.partition_broadcast(rbA[:], recipA[:])
            nc.vector.tensor_tensor(
                ctxT[0:64, hc, qw * 512 : (qw + 1) * 512],
                pcsA[:],
                rbA[:],
                OP.mult,
            )
            # ---- odd head: scratch + partition-shift DMA ----
            pcsB = cnpool.tile([64, 512], F32, tag="pcs", name="pcsB")
            nc.vector.tensor_copy(pcsB[:], pcB[0:64, :])
            sumB = rpool.tile([1, 512], F32, tag="rp", name="sumB")
            nc.vector.tensor_copy(sumB[:], pcB[64:65, :])
            recipB = rpool.tile([1, 512], F32, tag="rp", name="recipB")
            nc.vector.reciprocal_approx_fast(recipB[:], sumB[:])
            rbB = rbpool.tile([64, 512], F32, tag="rb", name="rbB")
            nc.gpsimd.partition_broadcast(rbB[:], recipB[:])
            cnB = cnpool.tile([64, 512], BF16, tag="cn", name="cnB")
            nc.vector.tensor_tensor(cnB[:], pcsB[:], rbB[:], OP.mult)
            nc.sync.dma_start(
                ctxT[64:128, hc, qw * 512 : (qw + 1) * 512], cnB[:]
            )

    # ---- phase 3: remaining output projection (qt 4-7 need qw1 ctxT) ----
    for qt in range(4, SC):
        for dh in range(2):
            outproj_step(qt, dh)()


def _get_nc():
    global _nc_cache
    if _nc_cache is None:
        _nc_cache = _build_nc()
    return _nc_cache


_nc_bench_cache = {}


def _get_bench_nc(repeat):
    if repeat not in _nc_bench_cache:
        _nc_bench_cache[repeat] = _build_nc(repeat)
    return _nc_bench_cache[repeat]


def _prep_in_maps(input_tensor, input_mask, Wq, bq, Wk, bk, Wv, bv, Wo, bo):
    bf16 = ml_dtypes.bfloat16
    x = np.ascontiguousarray(np.asarray(input_tensor, dtype=np.float32))
    mask = np.asarray(input_mask).astype(bool)
    Wq = np.asarray(Wq, dtype=np.float32).reshape(D, HK)
    Wk = np.asarray(Wk, dtype=np.float32).reshape(D, HK)
    Wv = np.asarray(Wv, dtype=np.float32).reshape(D, HK)
    Wo = np.asarray(Wo, dtype=np.float32).reshape(HK, D)
    bq = np.asarray(bq, dtype=np.float32).reshape(HK)
    bk = np.asarray(bk, dtype=np.float32).reshape(HK)
    bv = np.asarray(bv, dtype=np.float32).reshape(HK)
    bo = np.asarray(bo, dtype=np.float32).reshape(D)

    # fold the 1/sqrt(K)=1/8 score scale into Wq/bq (exact: power of two)
    wqs = np.ascontiguousarray((Wq / 8.0).astype(bf16))
    bqs = bq / 8.0
    wkb = np.ascontiguousarray(Wk.astype(bf16))
    wvb = np.ascontiguousarray(Wv.astype(bf16))
    wob = np.ascontiguousarray(Wo.astype(bf16))

    mf = mask.astype(np.float32)
    ka = (mf - 1.0) * 1e9   # 0 where kept, -1e9 where masked
    oneb = np.ones(1, bf16)

    def perm(v):
        # [n*128] chunk-major -> per-partition-contiguous [(p c)] layout
        return np.ascontiguousarray(v.reshape(-1, P).T).reshape(-1)

    bqp = perm(bqs)
    bkp = perm(bk)

    xb = x.astype(bf16)

    in_maps = []
    for b in range(B):
        in_maps.append(
            {
                "x": np.ascontiguousarray(xb[b]),
                "wq": wqs,
                "wk": wkb,
                "wv": wvb,
                "wo": wob,
                "bq": bqp,
                "bk": bkp,
                "bv": np.ascontiguousarray(bv),
                "bo": np.ascontiguousarray(bo),
                "ka": perm(ka[b]),
                "mq": perm(mf[b]),
                "oneb": oneb,
            }
        )
    return in_maps


def kernel(input_tensor, input_mask, Wq, bq, Wk, bk, Wv, bv, Wo, bo):
    in_maps = _prep_in_maps(
        input_tensor, input_mask, Wq, bq, Wk, bk, Wv, bv, Wo, bo
    )
    nc = _get_nc()
    res = run_bass_kernel_spmd(nc, in_maps, core_ids=list(range(B)), trace=TRACE)
    if TRACE:
        kernel.last_result = res
    out = np.stack([r["out"] for r in res.results], axis=0)
    return out


# revision 35
# speedup vs baseline: 1.2074x; 1.0373x over previous
"""Trainium2 Bass kernel for CustomSelfAttention (B=8,S=1024,D=1024,H=16,K=64).

Strategy: data-parallel over batch across 8 NeuronCores (1 batch item/core).
All matmuls in bf16 (host pre-casts x and weights; 1 cycle/column on the PE
vs 2 for fp32, FWL weight loads). Per-core pipeline:
  0. xT [d, s] via hardware XBAR DMA-transpose of bf16 x (no PE transposes).
  1. qT = (Wq/8)^T x^T, kT = Wk^T x^T (layout [hk, s]); v = x Wv stored
     interleaved with a ones column per head: vext [s, h, 65].
  2. attention per head PAIR (even head on PE rows 0-63, odd on 64-127 via
     tile_position row tiling -> the two K=64 scores matmuls run
     concurrently); one Exp activation over a 2-bank PSUM tile [128,1024]
     with the per-partition key-mask bias fused; ctx matmuls with
     lhsT=[v_h | 1] give ctx^T[k,q] plus softmax row sums in one shot;
     normalize with reciprocal_approx_fast + partition_broadcast.
     QKV projection matmuls for chunk c+1 are interleaved into attention
     chunk c's PE queue to cover the ACT-bound exp latency.
  3. out = ctxT^T Wo + bo, blended with the uniform-attention row for
     fully-masked queries (softmax of a row of -1e9 is exactly uniform),
     computed as u = mean_s(v) Wo.
"""

import contextlib
import sys
import types

sys.path.insert(0, "/opt/trn_rl_repo")

# The image's antenv package may lack axon_hooks (NTFF profile hook
# registry); bass_utils imports it unconditionally when trace=True.
# Install a functional shim + register the ctypes hook like
# trn_agent_boot.trn_boot does.
try:
    import antenv.axon_hooks  # noqa: F401
except ImportError:
    try:
        import antenv

        _hooks_mod = types.ModuleType("antenv.axon_hooks")
        _hook_box = [None]
        _hooks_mod.get_axon_ntff_profile_hook = lambda: _hook_box[0]
        _hooks_mod.set_axon_ntff_profile_hook = (
            lambda h: _hook_box.__setitem__(0, h)
        )
        sys.modules["antenv.axon_hooks"] = _hooks_mod
        antenv.axon_hooks = _hooks_mod
        from trn_agent_boot.trn_boot import _ntff_profile_via_ctypes

        _hooks_mod.set_axon_ntff_profile_hook(
            _ntff_profile_via_ctypes("/opt/axon/libaxon_pjrt.so")
        )
    except Exception:
        pass

import ml_dtypes  # noqa: E402
import numpy as np  # noqa: E402

import concourse.bass as bass  # noqa: E402
import concourse.bass_utils as _bass_utils  # noqa: E402
import concourse.mybir as mybir  # noqa: E402
import concourse.tile as tile  # noqa: E402
from concourse import bacc  # noqa: E402
from concourse.bass_utils import run_bass_kernel_spmd  # noqa: E402
from concourse.masks import make_identity  # noqa: E402

# Enable the walrus LDWEIGHTS background-buffer optimization for this
# kernel's compile: without it every MATMUL serializes behind its
# foreground weight load (~+170ns per matmul on this kernel). Walrus
# rejects ldw-opt when LDWEIGHTS carry semaphore waits, so the bass pass
# that moves matmul waits onto LDWEIGHTS must be skipped too (see
# _build_nc).
LDW_OPT = False

if not getattr(_bass_utils, "_ldwopt_patched", False):
    _orig_run_command = _bass_utils.run_command

    def _run_command_ldwopt(argv, **kwargs):
        if LDW_OPT and isinstance(argv, list):
            argv = [
                "--enable-ldw-opt=true" if a == "--enable-ldw-opt=false" else a
                for a in argv
            ]
        return _orig_run_command(argv, **kwargs)

    _bass_utils.run_command = _run_command_ldwopt
    _bass_utils._ldwopt_patched = True

F32 = mybir.dt.float32
BF16 = mybir.dt.bfloat16
AF = mybir.ActivationFunctionType
OP = mybir.AluOpType

B, S, D, H, K = 8, 1024, 1024, 16, 64
HK = H * K
P = 128
SC = S // P      # 8 s-chunks
DC = D // P      # 8 d-chunks
HKC = HK // P    # 8 hk-chunks (head pairs)
NQW = S // 512   # 2 q-windows of 512
NEG = -1e9

TRACE = False  # set by test.py for profiling runs

_nc_cache = None


def _build_nc(repeat=1):
    nc = bacc.Bacc(None, target_bir_lowering=False)
    if LDW_OPT:
        # leave waits on the matmuls; walrus ldw-opt refuses LDWEIGHTS
        # that carry semaphore waits
        nc.move_matmul_waits_to_ldweights = lambda: None

    x_d = nc.dram_tensor("x", [S, D], BF16, kind="ExternalInput")
    wq_d = nc.dram_tensor("wq", [D, HK], BF16, kind="ExternalInput")
    wk_d = nc.dram_tensor("wk", [D, HK], BF16, kind="ExternalInput")
    wv_d = nc.dram_tensor("wv", [D, HK], BF16, kind="ExternalInput")
    wo_d = nc.dram_tensor("wo", [HK, D], BF16, kind="ExternalInput")
    bq_d = nc.dram_tensor("bq", [HK], F32, kind="ExternalInput")
    bk_d = nc.dram_tensor("bk", [HK], F32, kind="ExternalInput")
    bv_d = nc.dram_tensor("bv", [HK], F32, kind="ExternalInput")
    bo_d = nc.dram_tensor("bo", [D], F32, kind="ExternalInput")
    ka_d = nc.dram_tensor("ka", [S], F32, kind="ExternalInput")   # (m-1)*1e9
    mq_d = nc.dram_tensor("mq", [S], F32, kind="ExternalInput")   # mask 0/1
    oneb_d = nc.dram_tensor("oneb", [1], BF16, kind="ExternalInput")
    out_d = nc.dram_tensor("out", [S, D], F32, kind="ExternalOutput")

    def bcast_ap(t, counts, step_last=None):
        # DRAM AP broadcasting a small tensor across leading 0-stride dims.
        ap = [[0, c] for c in counts]
        ap.append(step_last if step_last is not None else [1, 1])
        return bass.AP(tensor=t, offset=0, ap=ap)

    with tile.TileContext(nc) as tc:
        with (
            tc.tile_pool(name="consts", bufs=1) as consts,
            tc.tile_pool(name="big", bufs=1) as big,
            tc.tile_pool(name="wpool", bufs=1) as wpool,
            tc.tile_pool(name="epool", bufs=6) as epool,
            tc.tile_pool(name="rb", bufs=4) as rbpool,
            tc.tile_pool(name="rp", bufs=4) as rpool,
            tc.tile_pool(name="cn", bufs=4) as cnpool,
            tc.tile_pool(name="op", bufs=2) as opool,
            tc.tile_pool(name="dram", bufs=1, space="DRAM") as drampool,
            tc.tile_pool(name="pmm", bufs=2, space="PSUM") as pmm,
            tc.tile_pool(name="pscore", bufs=2, space="PSUM") as pscore,
            tc.tile_pool(name="pctx", bufs=2, space="PSUM") as pctx,
        ):
            # ---- constant tiles (DMAs emitted in _emit_body AFTER the
            # x transposes so they don't block the SP queue at t=0) ----
            ka_sb = consts.tile([P, SC], F32)
            mq_sb = consts.tile([P, SC], F32)
            bq_sb = consts.tile([P, HKC], F32)
            bk_sb = consts.tile([P, HKC], F32)
            bv_bc = consts.tile([P, HK], F32)
            bo_bc = consts.tile([P, D], F32)
            ones_col = consts.tile([P, 1], BF16)

            loop_cm = (
                tc.For_i(
                    0,
                    repeat,
                    1,
                    hint_engines=(
                        mybir.EngineType.PE,
                        mybir.EngineType.Activation,
                        mybir.EngineType.DVE,
                        mybir.EngineType.SP,
                        mybir.EngineType.Pool,
                    ),
                )
                if repeat > 1
                else contextlib.nullcontext()
            )
            with loop_cm:
                _emit_body(
                    nc, tc, x_d, wq_d, wk_d, wv_d, wo_d, out_d, bcast_ap,
                    oneb_d, ka_sb, mq_sb, bq_sb, bk_sb, bv_bc, bo_bc,
                    ones_col, consts, big, wpool, epool, rbpool, rpool,
                    cnpool, opool, drampool, pmm, pscore, pctx,
                    ka_d, mq_d, bq_d, bk_d, bv_d, bo_d,
                )

    nc.compile()
    return nc


def _emit_body(
    nc, tc, x_d, wq_d, wk_d, wv_d, wo_d, out_d, bcast_ap, oneb_d,
    ka_sb, mq_sb, bq_sb, bk_sb, bv_bc, bo_bc, ones_col, consts, big,
    wpool, epool, rbpool, rpool, cnpool, opool, drampool, pmm, pscore, pctx,
    ka_d, mq_d, bq_d, bk_d, bv_d, bo_d,
):
    # ---- persistent big tensors (all bf16) ----
    xT = big.tile([P, DC * S], BF16, tag="xT", name="xT").rearrange(
        "p (c s) -> p c s", c=DC
    )
    qT = big.tile([P, HKC * S], BF16, tag="qT", name="qT").rearrange(
        "p (c s) -> p c s", c=HKC
    )
    kT = big.tile([P, HKC * S], BF16, tag="kT", name="kT").rearrange(
        "p (c s) -> p c s", c=HKC
    )
    vext = big.tile([P, SC * H * (K + 1)], BF16, tag="vext", name="vext").rearrange(
        "p (s h k) -> p s h k", s=SC, h=H
    )
    ctxT = big.tile([P, HKC * S], BF16, tag="ctxT", name="ctxT").rearrange(
        "p (c s) -> p c s", c=HKC
    )
    # full-row weight layouts [p = row%128, chunk = row//128, 1024] (2KB lines)
    wqs = wpool.tile([P, DC * HK], BF16, tag="wq", name="wqs").rearrange(
        "p (c m) -> p c m", c=DC
    )
    wks = wpool.tile([P, DC * HK], BF16, tag="wk", name="wks").rearrange(
        "p (c m) -> p c m", c=DC
    )
    wvs = wpool.tile([P, DC * HK], BF16, tag="wv", name="wvs").rearrange(
        "p (c m) -> p c m", c=DC
    )
    wos = wpool.tile([P, HKC * D], BF16, tag="wo", name="wos").rearrange(
        "p (c m) -> p c m", c=HKC
    )

    # ---- phase 0: x -> xT via hardware XBAR DMA transpose (SP queue,
    # first in the queue so the v projection can start ASAP) ----
    # first 3 transpose pairs on SP; last pair on the ACT queue BEHIND
    # wvs (scalar is idle after wvs while SP still has ~5us of transposes)
    for dc2 in range(3):
        nc.sync.dma_start(
            xT[:, 2 * dc2 : 2 * dc2 + 2, :],
            x_d.ap()[:, dc2 * 256 : (dc2 + 1) * 256],
            transpose=True,
        )
    # weight loads on the ACT hardware DGE queue (parallel with SP)
    nc.scalar.dma_start(wvs[:], wv_d.ap().rearrange("(c p) m -> p c m", p=P))
    nc.scalar.dma_start(
        xT[:, 6:8, :],
        x_d.ap()[:, 768:1024],
        transpose=True,
    )
    nc.scalar.dma_start(wqs[:], wq_d.ap().rearrange("(c p) m -> p c m", p=P))
    nc.scalar.dma_start(wks[:], wk_d.ap().rearrange("(c p) m -> p c m", p=P))
    nc.scalar.dma_start(wos[:], wo_d.ap().rearrange("(c p) m -> p c m", p=P))

    # ones column of vext via Pool-engine memset (a broadcast DMA here
    # generates 16K 2-byte descriptors and stalls the SP queue for >100us)
    nc.gpsimd.memset(
        vext[:, :, :, K : K + 1].rearrange("p a b o -> p (a b) o"), 1.0
    )

    # constants: small DMAs behind the x transposes on the SP queue
    nc.sync.dma_start(bv_bc[:], bcast_ap(bv_d, [P], [1, HK]))
    nc.sync.dma_start(bq_sb[:], bq_d.ap().rearrange("(p c) -> p c", p=P))
    nc.sync.dma_start(bk_sb[:], bk_d.ap().rearrange("(p c) -> p c", p=P))
    nc.sync.dma_start(ka_sb[:], ka_d.ap().rearrange("(p c) -> p c", p=P))
    nc.sync.dma_start(mq_sb[:], mq_d.ap().rearrange("(p c) -> p c", p=P))
    nc.sync.dma_start(bo_bc[:], bcast_ap(bo_d, [P], [1, D]))
    nc.sync.dma_start(ones_col[:], bcast_ap(oneb_d, [P]))

    # ---- phase 1a: v projection into vext ----
    for hh in range(2):  # hk halves of 512
        for st in range(SC):
            ps = pmm.tile([P, 512], F32, tag="mm", name="ps")
            for dc in range(DC):
                nc.tensor.matmul(
                    ps[:],
                    xT[:, dc, st * P : (st + 1) * P],
                    wvs[:, dc, hh * 512 : (hh + 1) * 512],
                    start=(dc == 0),
                    stop=(dc == DC - 1),
                )
            nc.vector.tensor_tensor(
                vext[:, st, hh * 8 : (hh + 1) * 8, 0:K],
                ps[:].rearrange("p (h k) -> p h k", k=K),
                bv_bc[:, hh * 512 : (hh + 1) * 512].rearrange(
                    "p (h k) -> p h k", k=K
                ),
                OP.add,
            )

    # ---- qk projection steps (emitted interleaved with attention) ----
    # matmul computes lhsT.T @ rhs: for qT [hk, s] use lhsT = W chunk
    # [d, hk-cols], rhs = xT [d, s].
    def proj_chunk_steps(hkc):
        steps = []
        for w_sb, b_sb, dst in ((wqs, bq_sb, qT), (wks, bk_sb, kT)):
            for qw in range(NQW):
                ps_box = []

                def alloc(ps_box=ps_box):
                    ps_box.append(pmm.tile([P, 512], F32, tag="mm", name="ps"))

                steps.append(alloc)
                for dc in range(DC):
                    def mm(dc=dc, w_sb=w_sb, qw=qw, hkc=hkc, ps_box=ps_box):
                        nc.tensor.matmul(
                            ps_box[0][:],
                            w_sb[:, dc, hkc * P : (hkc + 1) * P],
                            xT[:, dc, qw * 512 : (qw + 1) * 512],
                            start=(dc == 0),
                            stop=(dc == DC - 1),
                        )
                    steps.append(mm)

                def bias(b_sb=b_sb, dst=dst, qw=qw, hkc=hkc, ps_box=ps_box):
                    nc.vector.tensor_scalar_add(
                        dst[:, hkc, qw * 512 : (qw + 1) * 512],
                        ps_box[0][:],
                        b_sb[:, hkc : hkc + 1],
                    )
                steps.append(bias)
        return steps

    # ---- u-path steps (uniform-attention fixup), emitted during chunk 7 ----
    mvh = consts.tile([1, HK], BF16)
    mvT = consts.tile([P, HKC], BF16)
    u_bc = consts.tile([P, D], F32)

    def upath_steps():
        steps = []
        for hh in range(2):
            ps_box = []

            def alloc(ps_box=ps_box):
                ps_box.append(pmm.tile([P, 512], F32, tag="mm", name="ps"))

            steps.append(alloc)
            for sc in range(SC):
                def mm(sc=sc, hh=hh, ps_box=ps_box):
                    nc.tensor.matmul(
                        ps_box[0][0:1, :].rearrange("o (h k) -> o h k", k=K),
                        ones_col[:],
                        vext[:, sc, hh * 8 : (hh + 1) * 8, 0:K],
                        start=(sc == 0),
                        stop=(sc == SC - 1),
                    )
                steps.append(mm)

            def fin(hh=hh, ps_box=ps_box):
                nc.vector.tensor_scalar_mul(
                    mvh[0:1, hh * 512 : (hh + 1) * 512],
                    ps_box[0][0:1, :],
                    1.0 / S,
                )
            steps.append(fin)

        # transpose mvh [1, HK] -> mvT [128, HKC] via 8 tiny N=1 matmuls
        # (avoids a DRAM roundtrip + a 1024x2B-descriptor DMA)
        def loadmvT():
            pmv = pmm.tile([P, 512], F32, tag="mm", name="ps")
            for c in range(HKC):
                nc.tensor.matmul(
                    pmv[:, c : c + 1],
                    mvh[0:1, c * P : (c + 1) * P],
                    ones_col[0:1, :],
                    start=True,
                    stop=True,
                )
            nc.vector.tensor_copy(mvT[:], pmv[:, 0:HKC])
        steps.append(loadmvT)

        for dh in range(2):
            ps_box = []

            def alloc(ps_box=ps_box):
                ps_box.append(pmm.tile([P, 512], F32, tag="mm", name="ps"))

            steps.append(alloc)
            for c in range(HKC):
                def mm(c=c, dh=dh, ps_box=ps_box):
                    nc.tensor.matmul(
                        ps_box[0][0:1, :],
                        mvT[:, c : c + 1],
                        wos[:, c, dh * 512 : (dh + 1) * 512],
                        start=(c == 0),
                        stop=(c == HKC - 1),
                    )
                steps.append(mm)

            def fin(dh=dh, ps_box=ps_box):
                uh = rpool.tile([1, 512], F32, tag="rp", name="uh")
                nc.vector.tensor_copy(uh[:], ps_box[0][0:1, :])
                nc.gpsimd.partition_broadcast(
                    u_bc[:, dh * 512 : (dh + 1) * 512], uh[:]
                )
            steps.append(fin)

        def addbo():
            nc.vector.tensor_tensor(u_bc[:], u_bc[:], bo_bc[:], OP.add)
        steps.append(addbo)
        return steps

    # ---- output-projection step for one (qt, dh): 8 matmuls + blend ----
    # qt < 4 reads only the qw0 half of ctxT (query rows < 512), so those
    # chunks can interleave into chunk 7's qw1 attention — keeping the PE
    # busy across the attention->projection transition (otherwise a ~6us
    # PE gap lets the HAM clock-gate re-throttle to 1.2 GHz for the tail).
    def outproj_step(qt, dh):
        def f():
            po = pmm.tile([P, 512], F32, tag="mm", name="ps")
            for c in range(HKC):
                nc.tensor.matmul(
                    po[:],
                    ctxT[:, c, qt * P : (qt + 1) * P],
                    wos[:, c, dh * 512 : (dh + 1) * 512],
                    start=(c == 0),
                    stop=(c == HKC - 1),
                )
            # out = (po - (u+bo))*mq + (u+bo)
            ub = u_bc[:, dh * 512 : (dh + 1) * 512]
            t1 = opool.tile([P, 512], F32, tag="o1", name="t1")
            nc.vector.tensor_tensor(t1[:], po[:], ub, OP.subtract)
            nc.vector.scalar_tensor_tensor(
                t1[:], t1[:], mq_sb[:, qt : qt + 1], ub, OP.mult, OP.add
            )
            nc.sync.dma_start(
                out_d.ap()[
                    qt * P : (qt + 1) * P, dh * 512 : (dh + 1) * 512
                ],
                t1[:],
            )
        return f

    # ---- phase 1b: qk chunk 0 emitted directly ----
    for step in proj_chunk_steps(0):
        step()

    # ---- phase 2: attention per head pair, proj chunk hc+1 interleaved ----
    for hc in range(HKC):
        if hc + 1 < HKC:
            steps_all = proj_chunk_steps(hc + 1)
            half = (len(steps_all) + 1) // 2
            pending_by_qw = [steps_all[:half], steps_all[half:]]
        else:
            pending_by_qw = [
                upath_steps(),
                [outproj_step(qt, dh) for qt in range(4) for dh in range(2)],
            ]

        hA, hB = 2 * hc, 2 * hc + 1
        for qw in range(NQW):
            pending = pending_by_qw[qw]
            nslots = SC + 1
            per_slot = (len(pending) + nslots - 1) // nslots
            pi = 0
            pcA = pctx.tile([P, 512], F32, tag="ctx", name="pcA")
            pcB = pctx.tile([P, 512], F32, tag="ctx", name="pcB")
            exs = {}
            for sc in range(SC + 1):
                if sc < SC:
                    pss2 = pscore.tile([P, 1024], F32, tag="score", name="pss2")
                    nc.tensor.matmul(
                        pss2[:, 0:512],
                        kT[0:64, hc, sc * P : (sc + 1) * P],
                        qT[0:64, hc, qw * 512 : (qw + 1) * 512],
                        start=True,
                        stop=True,
                    )
                    nc.tensor.matmul(
                        pss2[:, 512:1024],
                        kT[64:128, hc, sc * P : (sc + 1) * P],
                        qT[64:128, hc, qw * 512 : (qw + 1) * 512],
                        start=True,
                        stop=True,
                    )
                    ex = epool.tile([P, 1024], BF16, tag="exp", name="ex")
                    nc.scalar.activation(
                        ex[:], pss2[:], AF.Exp,
                        bias=ka_sb[:, sc : sc + 1], scale=1.0,
                    )
                    exs[sc] = ex
                # interleaved proj/upath steps (cover ACT latency)
                for _ in range(per_slot):
                    if pi < len(pending):
                        pending[pi]()
                        pi += 1
                if sc >= 1:
                    exm = exs.pop(sc - 1)
                    nc.tensor.matmul(
                        pcA[0:65, :],
                        vext[:, sc - 1, hA, :],
                        exm[:, 0:512],
                        start=(sc - 1 == 0),
                        stop=(sc - 1 == SC - 1),
                    )
                    nc.tensor.matmul(
                        pcB[0:65, :],
                        vext[:, sc - 1, hB, :],
                        exm[:, 512:1024],
                        start=(sc - 1 == 0),
                        stop=(sc - 1 == SC - 1),
                    )
            while pi < len(pending):
                pending[pi]()
                pi += 1
            # ---- normalization (off the PSUM critical path): copy each
            # [65,512] accumulator to SBUF first — frees the PSUM bank for
            # the next q-window earlier, and feeds reciprocal_approx_fast
            # from SBUF (from PSUM the bit-trick seed reads garbage on HW).
            pcsA = cnpool.tile([64, 512], F32, tag="pcs", name="pcsA")
            nc.vector.tensor_copy(pcsA[:], pcA[0:64, :])
            sumA = rpool.tile([1, 512], F32, tag="rp", name="sumA")
            nc.vector.tensor_copy(sumA[:], pcA[64:65, :])
            recipA = rpool.tile([1, 512], F32, tag="rp", name="recipA")
            nc.vector.reciprocal_approx_fast(recipA[:], sumA[:])
            rbA = rbpool.tile([64, 512], F32, tag="rb", name="rbA")
            nc.gpsimd.partition_broadcast(rbA[:], recipA[:])
            nc.vector.tensor_tensor(
                ctxT[0:64, hc, qw * 512 : (qw + 1) * 512],
                pcsA[:],
                rbA[:],
                OP.mult,
            )
            # ---- odd head: scratch + partition-shift DMA ----
            pcsB = cnpool.tile([64, 512], F32, tag="pcs", name="pcsB")
            nc.vector.tensor_copy(pcsB[:], pcB[0:64, :])
            sumB = rpool.tile([1, 512], F32, tag="rp", name="sumB")
            nc.vector.tensor_copy(sumB[:], pcB[64:65, :])
            recipB = rpool.tile([1, 512], F32, tag="rp", name="recipB")
            nc.vector.reciprocal_approx_fast(recipB[:], sumB[:])
            rbB = rbpool.tile([64, 512], F32, tag="rb", name="rbB")
            nc.gpsimd.partition_broadcast(rbB[:], recipB[:])
            cnB = cnpool.tile([64, 512], BF16, tag="cn", name="cnB")
            nc.vector.tensor_tensor(cnB[:], pcsB[:], rbB[:], OP.mult)
            nc.sync.dma_start(
                ctxT[64:128, hc, qw * 512 : (qw + 1) * 512], cnB[:]
            )

    # ---- phase 3: remaining output projection (qt 4-7 need qw1 ctxT) ----
    for qt in range(4, SC):
        for dh in range(2):
            outproj_step(qt, dh)()


def _get_nc():
    global _nc_cache
    if _nc_cache is None:
        _nc_cache = _build_nc()
    return _nc_cache


_nc_bench_cache = {}


def _get_bench_nc(repeat):
    if repeat not in _nc_bench_cache:
        _nc_bench_cache[repeat] = _build_nc(repeat)
    return _nc_bench_cache[repeat]


def _prep_in_maps(input_tensor, input_mask, Wq, bq, Wk, bk, Wv, bv, Wo, bo):
    bf16 = ml_dtypes.bfloat16
    x = np.ascontiguousarray(np.asarray(input_tensor, dtype=np.float32))
    mask = np.asarray(input_mask).astype(bool)
    Wq = np.asarray(Wq, dtype=np.float32).reshape(D, HK)
    Wk = np.asarray(Wk, dtype=np.float32).reshape(D, HK)
    Wv = np.asarray(Wv, dtype=np.float32).reshape(D, HK)
    Wo = np.asarray(Wo, dtype=np.float32).reshape(HK, D)
    bq = np.asarray(bq, dtype=np.float32).reshape(HK)
    bk = np.asarray(bk, dtype=np.float32).reshape(HK)
    bv = np.asarray(bv, dtype=np.float32).reshape(HK)
    bo = np.asarray(bo, dtype=np.float32).reshape(D)

    # fold the 1/sqrt(K)=1/8 score scale into Wq/bq (exact: power of two)
    wqs = np.ascontiguousarray((Wq / 8.0).astype(bf16))
    bqs = bq / 8.0
    wkb = np.ascontiguousarray(Wk.astype(bf16))
    wvb = np.ascontiguousarray(Wv.astype(bf16))
    wob = np.ascontiguousarray(Wo.astype(bf16))

    mf = mask.astype(np.float32)
    ka = (mf - 1.0) * 1e9   # 0 where kept, -1e9 where masked
    oneb = np.ones(1, bf16)

    def perm(v):
        # [n*128] chunk-major -> per-partition-contiguous [(p c)] layout
        return np.ascontiguousarray(v.reshape(-1, P).T).reshape(-1)

    bqp = perm(bqs)
    bkp = perm(bk)

    xb = x.astype(bf16)

    in_maps = []
    for b in range(B):
        in_maps.append(
            {
                "x": np.ascontiguousarray(xb[b]),
                "wq": wqs,
                "wk": wkb,
                "wv": wvb,
                "wo": wob,
                "bq": bqp,
                "bk": bkp,
                "bv": np.ascontiguousarray(bv),
                "bo": np.ascontiguousarray(bo),
                "ka": perm(ka[b]),
                "mq": perm(mf[b]),
                "oneb": oneb,
            }
        )
    return in_maps


def kernel(input_tensor, input_mask, Wq, bq, Wk, bk, Wv, bv, Wo, bo):
    in_maps = _prep_in_maps(
        input_tensor, input_mask, Wq, bq, Wk, bk, Wv, bv, Wo, bo
    )
    nc = _get_nc()
    res = run_bass_kernel_spmd(nc, in_maps, core_ids=list(range(B)), trace=TRACE)
    if TRACE:
        kernel.last_result = res
    out = np.stack([r["out"] for r in res.results], axis=0)
    return out


# revision 36
# speedup vs baseline: 1.2275x; 1.0166x over previous
"""Trainium2 Bass kernel for CustomSelfAttention (B=8,S=1024,D=1024,H=16,K=64).

Strategy: data-parallel over batch across 8 NeuronCores (1 batch item/core).
All matmuls in bf16 (host pre-casts x and weights; 1 cycle/column on the PE
vs 2 for fp32, FWL weight loads). Per-core pipeline:
  0. xT [d, s] via hardware XBAR DMA-transpose of bf16 x (no PE transposes).
  1. qT = (Wq/8)^T x^T, kT = Wk^T x^T (layout [hk, s]); v = x Wv stored
     interleaved with a ones column per head: vext [s, h, 65].
  2. attention per head PAIR (even head on PE rows 0-63, odd on 64-127 via
     tile_position row tiling -> the two K=64 scores matmuls run
     concurrently); one Exp activation over a 2-bank PSUM tile [128,1024]
     with the per-partition key-mask bias fused; ctx matmuls with
     lhsT=[v_h | 1] give ctx^T[k,q] plus softmax row sums in one shot;
     normalize with reciprocal_approx_fast + partition_broadcast.
     QKV projection matmuls for chunk c+1 are interleaved into attention
     chunk c's PE queue to cover the ACT-bound exp latency.
  3. out = ctxT^T Wo + bo, blended with the uniform-attention row for
     fully-masked queries (softmax of a row of -1e9 is exactly uniform),
     computed as u = mean_s(v) Wo.
"""

import contextlib
import sys
import types

sys.path.insert(0, "/opt/trn_rl_repo")

# The image's antenv package may lack axon_hooks (NTFF profile hook
# registry); bass_utils imports it unconditionally when trace=True.
# Install a functional shim + register the ctypes hook like
# trn_agent_boot.trn_boot does.
try:
    import antenv.axon_hooks  # noqa: F401
except ImportError:
    try:
        import antenv

        _hooks_mod = types.ModuleType("antenv.axon_hooks")
        _hook_box = [None]
        _hooks_mod.get_axon_ntff_profile_hook = lambda: _hook_box[0]
        _hooks_mod.set_axon_ntff_profile_hook = (
            lambda h: _hook_box.__setitem__(0, h)
        )
        sys.modules["antenv.axon_hooks"] = _hooks_mod
        antenv.axon_hooks = _hooks_mod
        from trn_agent_boot.trn_boot import _ntff_profile_via_ctypes

        _hooks_mod.set_axon_ntff_profile_hook(
            _ntff_profile_via_ctypes("/opt/axon/libaxon_pjrt.so")
        )
    except Exception:
        pass

import ml_dtypes  # noqa: E402
import numpy as np  # noqa: E402

import concourse.bass as bass  # noqa: E402
import concourse.bass_utils as _bass_utils  # noqa: E402
import concourse.mybir as mybir  # noqa: E402
import concourse.tile as tile  # noqa: E402
from concourse import bacc  # noqa: E402
from concourse.bass_utils import run_bass_kernel_spmd  # noqa: E402
from concourse.masks import make_identity  # noqa: E402

# Enable the walrus LDWEIGHTS background-buffer optimization for this
# kernel's compile: without it every MATMUL serializes behind its
# foreground weight load (~+170ns per matmul on this kernel). Walrus
# rejects ldw-opt when LDWEIGHTS carry semaphore waits, so the bass pass
# that moves matmul waits onto LDWEIGHTS must be skipped too (see
# _build_nc).
LDW_OPT = False

if not getattr(_bass_utils, "_ldwopt_patched", False):
    _orig_run_command = _bass_utils.run_command

    def _run_command_ldwopt(argv, **kwargs):
        if LDW_OPT and isinstance(argv, list):
            argv = [
                "--enable-ldw-opt=true" if a == "--enable-ldw-opt=false" else a
                for a in argv
            ]
        return _orig_run_command(argv, **kwargs)

    _bass_utils.run_command = _run_command_ldwopt
    _bass_utils._ldwopt_patched = True

F32 = mybir.dt.float32
BF16 = mybir.dt.bfloat16
AF = mybir.ActivationFunctionType
OP = mybir.AluOpType

B, S, D, H, K = 8, 1024, 1024, 16, 64
HK = H * K
P = 128
SC = S // P      # 8 s-chunks
DC = D // P      # 8 d-chunks
HKC = HK // P    # 8 hk-chunks (head pairs)
NQW = S // 512   # 2 q-windows of 512
NEG = -1e9

TRACE = False  # set by test.py for profiling runs

_nc_cache = None


def _build_nc(repeat=1):
    nc = bacc.Bacc(None, target_bir_lowering=False)
    if LDW_OPT:
        # leave waits on the matmuls; walrus ldw-opt refuses LDWEIGHTS
        # that carry semaphore waits
        nc.move_matmul_waits_to_ldweights = lambda: None

    x_d = nc.dram_tensor("x", [S, D], BF16, kind="ExternalInput")
    wq_d = nc.dram_tensor("wq", [D, HK], BF16, kind="ExternalInput")
    wk_d = nc.dram_tensor("wk", [D, HK], BF16, kind="ExternalInput")
    wv_d = nc.dram_tensor("wv", [D, HK], BF16, kind="ExternalInput")
    wo_d = nc.dram_tensor("wo", [HK, D], BF16, kind="ExternalInput")
    bq_d = nc.dram_tensor("bq", [HK], F32, kind="ExternalInput")
    bk_d = nc.dram_tensor("bk", [HK], F32, kind="ExternalInput")
    bv_d = nc.dram_tensor("bv", [HK], F32, kind="ExternalInput")
    bo_d = nc.dram_tensor("bo", [D], F32, kind="ExternalInput")
    ka_d = nc.dram_tensor("ka", [S], F32, kind="ExternalInput")   # (m-1)*1e9
    mq_d = nc.dram_tensor("mq", [S], F32, kind="ExternalInput")   # mask 0/1
    oneb_d = nc.dram_tensor("oneb", [1], BF16, kind="ExternalInput")
    out_d = nc.dram_tensor("out", [S, D], F32, kind="ExternalOutput")

    def bcast_ap(t, counts, step_last=None):
        # DRAM AP broadcasting a small tensor across leading 0-stride dims.
        ap = [[0, c] for c in counts]
        ap.append(step_last if step_last is not None else [1, 1])
        return bass.AP(tensor=t, offset=0, ap=ap)

    with tile.TileContext(nc) as tc:
        with (
            tc.tile_pool(name="consts", bufs=1) as consts,
            tc.tile_pool(name="big", bufs=1) as big,
            tc.tile_pool(name="wpool", bufs=1) as wpool,
            tc.tile_pool(name="epool", bufs=6) as epool,
            tc.tile_pool(name="rb", bufs=2) as rbpool,
            tc.tile_pool(name="rp", bufs=4) as rpool,
            tc.tile_pool(name="cn", bufs=2) as cnpool,
            tc.tile_pool(name="op", bufs=2) as opool,
            tc.tile_pool(name="dram", bufs=1, space="DRAM") as drampool,
            tc.tile_pool(name="pmm", bufs=2, space="PSUM") as pmm,
            tc.tile_pool(name="pscore", bufs=2, space="PSUM") as pscore,
            tc.tile_pool(name="pctx", bufs=2, space="PSUM") as pctx,
        ):
            # ---- constant tiles (DMAs emitted in _emit_body AFTER the
            # x transposes so they don't block the SP queue at t=0) ----
            ka_sb = consts.tile([P, SC], F32)
            mq_sb = consts.tile([P, SC], F32)
            bq_sb = consts.tile([P, HKC], F32)
            bk_sb = consts.tile([P, HKC], F32)
            bv_bc = consts.tile([P, HK], F32)
            bo_bc = consts.tile([P, D], F32)
            ones_col = consts.tile([P, 1], BF16)

            loop_cm = (
                tc.For_i(
                    0,
                    repeat,
                    1,
                    hint_engines=(
                        mybir.EngineType.PE,
                        mybir.EngineType.Activation,
                        mybir.EngineType.DVE,
                        mybir.EngineType.SP,
                        mybir.EngineType.Pool,
                    ),
                )
                if repeat > 1
                else contextlib.nullcontext()
            )
            with loop_cm:
                _emit_body(
                    nc, tc, x_d, wq_d, wk_d, wv_d, wo_d, out_d, bcast_ap,
                    oneb_d, ka_sb, mq_sb, bq_sb, bk_sb, bv_bc, bo_bc,
                    ones_col, consts, big, wpool, epool, rbpool, rpool,
                    cnpool, opool, drampool, pmm, pscore, pctx,
                    ka_d, mq_d, bq_d, bk_d, bv_d, bo_d,
                )

    nc.compile()
    return nc


def _emit_body(
    nc, tc, x_d, wq_d, wk_d, wv_d, wo_d, out_d, bcast_ap, oneb_d,
    ka_sb, mq_sb, bq_sb, bk_sb, bv_bc, bo_bc, ones_col, consts, big,
    wpool, epool, rbpool, rpool, cnpool, opool, drampool, pmm, pscore, pctx,
    ka_d, mq_d, bq_d, bk_d, bv_d, bo_d,
):
    # ---- persistent big tensors (all bf16) ----
    xT = big.tile([P, DC * S], BF16, tag="xT", name="xT").rearrange(
        "p (c s) -> p c s", c=DC
    )
    qT = big.tile([P, HKC * S], BF16, tag="qT", name="qT").rearrange(
        "p (c s) -> p c s", c=HKC
    )
    kT = big.tile([P, HKC * S], BF16, tag="kT", name="kT").rearrange(
        "p (c s) -> p c s", c=HKC
    )
    vext = big.tile([P, SC * H * (K + 1)], BF16, tag="vext", name="vext").rearrange(
        "p (s h k) -> p s h k", s=SC, h=H
    )
    ctxT = big.tile([P, HKC * S], BF16, tag="ctxT", name="ctxT").rearrange(
        "p (c s) -> p c s", c=HKC
    )
    # full-row weight layouts [p = row%128, chunk = row//128, 1024] (2KB lines)
    wqs = wpool.tile([P, DC * HK], BF16, tag="wq", name="wqs").rearrange(
        "p (c m) -> p c m", c=DC
    )
    wks = wpool.tile([P, DC * HK], BF16, tag="wk", name="wks").rearrange(
        "p (c m) -> p c m", c=DC
    )
    wvs = wpool.tile([P, DC * HK], BF16, tag="wv", name="wvs").rearrange(
        "p (c m) -> p c m", c=DC
    )
    wos = wpool.tile([P, HKC * D], BF16, tag="wo", name="wos").rearrange(
        "p (c m) -> p c m", c=HKC
    )

    # ---- phase 0: x -> xT via hardware XBAR DMA transpose (SP queue,
    # first in the queue so the v projection can start ASAP) ----
    for dc2 in range(DC // 2):
        nc.sync.dma_start(
            xT[:, 2 * dc2 : 2 * dc2 + 2, :],
            x_d.ap()[:, dc2 * 256 : (dc2 + 1) * 256],
            transpose=True,
        )
    # weight loads on the ACT hardware DGE queue (parallel with SP)
    nc.scalar.dma_start(wvs[:], wv_d.ap().rearrange("(c p) m -> p c m", p=P))
    nc.scalar.dma_start(wqs[:], wq_d.ap().rearrange("(c p) m -> p c m", p=P))
    nc.scalar.dma_start(wks[:], wk_d.ap().rearrange("(c p) m -> p c m", p=P))
    nc.scalar.dma_start(wos[:], wo_d.ap().rearrange("(c p) m -> p c m", p=P))

    # ones column of vext via Pool-engine memset (a broadcast DMA here
    # generates 16K 2-byte descriptors and stalls the SP queue for >100us)
    nc.gpsimd.memset(
        vext[:, :, :, K : K + 1].rearrange("p a b o -> p (a b) o"), 1.0
    )

    # constants: small DMAs behind the x transposes on the SP queue
    nc.sync.dma_start(bv_bc[:], bcast_ap(bv_d, [P], [1, HK]))
    nc.sync.dma_start(bq_sb[:], bq_d.ap().rearrange("(p c) -> p c", p=P))
    nc.sync.dma_start(bk_sb[:], bk_d.ap().rearrange("(p c) -> p c", p=P))
    nc.sync.dma_start(ka_sb[:], ka_d.ap().rearrange("(p c) -> p c", p=P))
    nc.sync.dma_start(mq_sb[:], mq_d.ap().rearrange("(p c) -> p c", p=P))
    nc.sync.dma_start(bo_bc[:], bcast_ap(bo_d, [P], [1, D]))
    nc.sync.dma_start(ones_col[:], bcast_ap(oneb_d, [P]))

    # ---- phase 1a: v projection into vext ----
    for hh in range(2):  # hk halves of 512
        for st in range(SC):
            ps = pmm.tile([P, 512], F32, tag="mm", name="ps")
            for dc in range(DC):
                nc.tensor.matmul(
                    ps[:],
                    xT[:, dc, st * P : (st + 1) * P],
                    wvs[:, dc, hh * 512 : (hh + 1) * 512],
                    start=(dc == 0),
                    stop=(dc == DC - 1),
                )
            nc.vector.tensor_tensor(
                vext[:, st, hh * 8 : (hh + 1) * 8, 0:K],
                ps[:].rearrange("p (h k) -> p h k", k=K),
                bv_bc[:, hh * 512 : (hh + 1) * 512].rearrange(
                    "p (h k) -> p h k", k=K
                ),
                OP.add,
            )

    # ---- qk projection steps (emitted interleaved with attention) ----
    # matmul computes lhsT.T @ rhs: for qT [hk, s] use lhsT = W chunk
    # [d, hk-cols], rhs = xT [d, s].
    def proj_chunk_steps(hkc):
        steps = []
        for w_sb, b_sb, dst in ((wqs, bq_sb, qT), (wks, bk_sb, kT)):
            for qw in range(NQW):
                ps_box = []

                def alloc(ps_box=ps_box):
                    ps_box.append(pmm.tile([P, 512], F32, tag="mm", name="ps"))

                steps.append(alloc)
                for dc in range(DC):
                    def mm(dc=dc, w_sb=w_sb, qw=qw, hkc=hkc, ps_box=ps_box):
                        nc.tensor.matmul(
                            ps_box[0][:],
                            w_sb[:, dc, hkc * P : (hkc + 1) * P],
                            xT[:, dc, qw * 512 : (qw + 1) * 512],
                            start=(dc == 0),
                            stop=(dc == DC - 1),
                        )
                    steps.append(mm)

                def bias(b_sb=b_sb, dst=dst, qw=qw, hkc=hkc, ps_box=ps_box):
                    nc.vector.tensor_scalar_add(
                        dst[:, hkc, qw * 512 : (qw + 1) * 512],
                        ps_box[0][:],
                        b_sb[:, hkc : hkc + 1],
                    )
                steps.append(bias)
        return steps

    # ---- u-path steps (uniform-attention fixup), emitted during chunk 7 ----
    mvh = consts.tile([1, HK], BF16)
    mvT = consts.tile([P, HKC], BF16)
    u_bc = consts.tile([P, D], F32)

    def upath_steps():
        steps = []
        for hh in range(2):
            ps_box = []

            def alloc(ps_box=ps_box):
                ps_box.append(pmm.tile([P, 512], F32, tag="mm", name="ps"))

            steps.append(alloc)
            for sc in range(SC):
                def mm(sc=sc, hh=hh, ps_box=ps_box):
                    nc.tensor.matmul(
                        ps_box[0][0:1, :].rearrange("o (h k) -> o h k", k=K),
                        ones_col[:],
                        vext[:, sc, hh * 8 : (hh + 1) * 8, 0:K],
                        start=(sc == 0),
                        stop=(sc == SC - 1),
                    )
                steps.append(mm)

            def fin(hh=hh, ps_box=ps_box):
                nc.vector.tensor_scalar_mul(
                    mvh[0:1, hh * 512 : (hh + 1) * 512],
                    ps_box[0][0:1, :],
                    1.0 / S,
                )
            steps.append(fin)

        # transpose mvh [1, HK] -> mvT [128, HKC] via 8 tiny N=1 matmuls
        # (avoids a DRAM roundtrip + a 1024x2B-descriptor DMA)
        def loadmvT():
            pmv = pmm.tile([P, 512], F32, tag="mm", name="ps")
            for c in range(HKC):
                nc.tensor.matmul(
                    pmv[:, c : c + 1],
                    mvh[0:1, c * P : (c + 1) * P],
                    ones_col[0:1, :],
                    start=True,
                    stop=True,
                )
            nc.vector.tensor_copy(mvT[:], pmv[:, 0:HKC])
        steps.append(loadmvT)

        for dh in range(2):
            ps_box = []

            def alloc(ps_box=ps_box):
                ps_box.append(pmm.tile([P, 512], F32, tag="mm", name="ps"))

            steps.append(alloc)
            for c in range(HKC):
                def mm(c=c, dh=dh, ps_box=ps_box):
                    nc.tensor.matmul(
                        ps_box[0][0:1, :],
                        mvT[:, c : c + 1],
                        wos[:, c, dh * 512 : (dh + 1) * 512],
                        start=(c == 0),
                        stop=(c == HKC - 1),
                    )
                steps.append(mm)

            def fin(dh=dh, ps_box=ps_box):
                uh = rpool.tile([1, 512], F32, tag="rp", name="uh")
                nc.vector.tensor_copy(uh[:], ps_box[0][0:1, :])
                nc.gpsimd.partition_broadcast(
                    u_bc[:, dh * 512 : (dh + 1) * 512], uh[:]
                )
            steps.append(fin)

        def addbo():
            nc.vector.tensor_tensor(u_bc[:], u_bc[:], bo_bc[:], OP.add)
        steps.append(addbo)
        return steps

    # ---- output-projection step for one (qt, dh): 8 matmuls + blend ----
    # qt < 4 reads only the qw0 half of ctxT (query rows < 512), so those
    # chunks can interleave into chunk 7's qw1 attention — keeping the PE
    # busy across the attention->projection transition (otherwise a ~6us
    # PE gap lets the HAM clock-gate re-throttle to 1.2 GHz for the tail).
    def outproj_step(qt, dh):
        def f():
            po = pmm.tile([P, 512], F32, tag="mm", name="ps")
            for c in range(HKC):
                nc.tensor.matmul(
                    po[:],
                    ctxT[:, c, qt * P : (qt + 1) * P],
                    wos[:, c, dh * 512 : (dh + 1) * 512],
                    start=(c == 0),
                    stop=(c == HKC - 1),
                )
            # out = (po - (u+bo))*mq + (u+bo)
            ub = u_bc[:, dh * 512 : (dh + 1) * 512]
            t1 = opool.tile([P, 512], F32, tag="o1", name="t1")
            nc.vector.tensor_tensor(t1[:], po[:], ub, OP.subtract)
            nc.vector.scalar_tensor_tensor(
                t1[:], t1[:], mq_sb[:, qt : qt + 1], ub, OP.mult, OP.add
            )
            nc.sync.dma_start(
                out_d.ap()[
                    qt * P : (qt + 1) * P, dh * 512 : (dh + 1) * 512
                ],
                t1[:],
            )
        return f

    # ---- phase 1b: qk chunk 0 emitted directly ----
    for step in proj_chunk_steps(0):
        step()

    # ---- phase 2: attention per head pair, proj chunk hc+1 interleaved ----
    for hc in range(HKC):
        if hc + 1 < HKC:
            steps_all = proj_chunk_steps(hc + 1)
            half = (len(steps_all) + 1) // 2
            pending_by_qw = [steps_all[:half], steps_all[half:]]
        else:
            pending_by_qw = [
                upath_steps(),
                [outproj_step(qt, dh) for qt in range(4) for dh in range(2)],
            ]

        hA, hB = 2 * hc, 2 * hc + 1
        for qw in range(NQW):
            pending = pending_by_qw[qw]
            nslots = SC + 1
            per_slot = (len(pending) + nslots - 1) // nslots
            pi = 0
            pcA = pctx.tile([P, 512], F32, tag="ctx", name="pcA")
            pcB = pctx.tile([P, 512], F32, tag="ctx", name="pcB")
            exs = {}
            for sc in range(SC + 1):
                if sc < SC:
                    pss2 = pscore.tile([P, 1024], F32, tag="score", name="pss2")
                    nc.tensor.matmul(
                        pss2[:, 0:512],
                        kT[0:64, hc, sc * P : (sc + 1) * P],
                        qT[0:64, hc, qw * 512 : (qw + 1) * 512],
                        start=True,
                        stop=True,
                    )
                    nc.tensor.matmul(
                        pss2[:, 512:1024],
                        kT[64:128, hc, sc * P : (sc + 1) * P],
                        qT[64:128, hc, qw * 512 : (qw + 1) * 512],
                        start=True,
                        stop=True,
                    )
                    ex = epool.tile([P, 1024], BF16, tag="exp", name="ex")
                    nc.scalar.activation(
                        ex[:], pss2[:], AF.Exp,
                        bias=ka_sb[:, sc : sc + 1], scale=1.0,
                    )
                    exs[sc] = ex
                # interleaved proj/upath steps (cover ACT latency)
                for _ in range(per_slot):
                    if pi < len(pending):
                        pending[pi]()
                        pi += 1
                if sc >= 1:
                    exm = exs.pop(sc - 1)
                    nc.tensor.matmul(
                        pcA[0:65, :],
                        vext[:, sc - 1, hA, :],
                        exm[:, 0:512],
                        start=(sc - 1 == 0),
                        stop=(sc - 1 == SC - 1),
                    )
                    nc.tensor.matmul(
                        pcB[0:65, :],
                        vext[:, sc - 1, hB, :],
                        exm[:, 512:1024],
                        start=(sc - 1 == 0),
                        stop=(sc - 1 == SC - 1),
                    )
            while pi < len(pending):
                pending[pi]()
                pi += 1
            # ---- normalization (off the PSUM critical path): copy each
            # [65,512] accumulator to SBUF first — frees the PSUM bank for
            # the next q-window earlier, and feeds reciprocal_approx_fast
            # from SBUF (from PSUM the bit-trick seed reads garbage on HW).
            pcsA = cnpool.tile([64, 512], F32, tag="pcs", name="pcsA")
            nc.vector.tensor_copy(pcsA[:], pcA[0:64, :])
            sumA = rpool.tile([1, 512], F32, tag="rp", name="sumA")
            nc.vector.tensor_copy(sumA[:], pcA[64:65, :])
            recipA = rpool.tile([1, 512], F32, tag="rp", name="recipA")
            nc.vector.reciprocal_approx_fast(recipA[:], sumA[:])
            rbA = rbpool.tile([64, 512], F32, tag="rb", name="rbA")
            nc.gpsimd.partition_broadcast(rbA[:], recipA[:])
            nc.vector.tensor_tensor(
                ctxT[0:64, hc, qw * 512 : (qw + 1) * 512],
                pcsA[:],
                rbA[:],
                OP.mult,
            )
            # ---- odd head: scratch + partition-shift DMA ----
            pcsB = cnpool.tile([64, 512], F32, tag="pcs", name="pcsB")
            nc.vector.tensor_copy(pcsB[:], pcB[0:64, :])
            sumB = rpool.tile([1, 512], F32, tag="rp", name="sumB")
            nc.vector.tensor_copy(sumB[:], pcB[64:65, :])
            recipB = rpool.tile([1, 512], F32, tag="rp", name="recipB")
            nc.vector.reciprocal_approx_fast(recipB[:], sumB[:])
            rbB = rbpool.tile([64, 512], F32, tag="rb", name="rbB")
            nc.gpsimd.partition_broadcast(rbB[:], recipB[:])
            cnB = cnpool.tile([64, 512], BF16, tag="cn", name="cnB")
            nc.vector.tensor_tensor(cnB[:], pcsB[:], rbB[:], OP.mult)
            nc.sync.dma_start(
                ctxT[64:128, hc, qw * 512 : (qw + 1) * 512], cnB[:]
            )

    # ---- phase 3: remaining output projection (qt 4-7 need qw1 ctxT) ----
    for qt in range(4, SC):
        for dh in range(2):
            outproj_step(qt, dh)()


def _get_nc():
    global _nc_cache
    if _nc_cache is None:
        _nc_cache = _build_nc()
    return _nc_cache


_nc_bench_cache = {}


def _get_bench_nc(repeat):
    if repeat not in _nc_bench_cache:
        _nc_bench_cache[repeat] = _build_nc(repeat)
    return _nc_bench_cache[repeat]


def _prep_in_maps(input_tensor, input_mask, Wq, bq, Wk, bk, Wv, bv, Wo, bo):
    bf16 = ml_dtypes.bfloat16
    x = np.ascontiguousarray(np.asarray(input_tensor, dtype=np.float32))
    mask = np.asarray(input_mask).astype(bool)
    Wq = np.asarray(Wq, dtype=np.float32).reshape(D, HK)
    Wk = np.asarray(Wk, dtype=np.float32).reshape(D, HK)
    Wv = np.asarray(Wv, dtype=np.float32).reshape(D, HK)
    Wo = np.asarray(Wo, dtype=np.float32).reshape(HK, D)
    bq = np.asarray(bq, dtype=np.float32).reshape(HK)
    bk = np.asarray(bk, dtype=np.float32).reshape(HK)
    bv = np.asarray(bv, dtype=np.float32).reshape(HK)
    bo = np.asarray(bo, dtype=np.float32).reshape(D)

    # fold the 1/sqrt(K)=1/8 score scale into Wq/bq (exact: power of two)
    wqs = np.ascontiguousarray((Wq / 8.0).astype(bf16))
    bqs = bq / 8.0
    wkb = np.ascontiguousarray(Wk.astype(bf16))
    wvb = np.ascontiguousarray(Wv.astype(bf16))
    wob = np.ascontiguousarray(Wo.astype(bf16))

    mf = mask.astype(np.float32)
    ka = (mf - 1.0) * 1e9   # 0 where kept, -1e9 where masked
    oneb = np.ones(1, bf16)

    def perm(v):
        # [n*128] chunk-major -> per-partition-contiguous [(p c)] layout
        return np.ascontiguousarray(v.reshape(-1, P).T).reshape(-1)

    bqp = perm(bqs)
    bkp = perm(bk)

    xb = x.astype(bf16)

    in_maps = []
    for b in range(B):
        in_maps.append(
            {
                "x": np.ascontiguousarray(xb[b]),
                "wq": wqs,
                "wk": wkb,
                "wv": wvb,
                "wo": wob,
                "bq": bqp,
                "bk": bkp,
                "bv": np.ascontiguousarray(bv),
                "bo": np.ascontiguousarray(bo),
                "ka": perm(ka[b]),
                "mq": perm(mf[b]),
                "oneb": oneb,
            }
        )
    return in_maps


def kernel(input_tensor, input_mask, Wq, bq, Wk, bk, Wv, bv, Wo, bo):
    in_maps = _prep_in_maps(
        input_tensor, input_mask, Wq, bq, Wk, bk, Wv, bv, Wo, bo
    )
    nc = _get_nc()
    res = run_bass_kernel_spmd(nc, in_maps, core_ids=list(range(B)), trace=TRACE)
    if TRACE:
        kernel.last_result = res
    out = np.stack([r["out"] for r in res.results], axis=0)
    return out
